# revision 3
# baseline (speedup 1.0000x reference)
"""Trainium2 Bass kernel for nn_CognitiveManifold (geodesic RK2 step).

8 NeuronCores, pure data parallel: 8192 tokens/core, full inputs in,
full outputs out. Analytic metric derivatives (matches the reference's
eps=1e-4 central FD to ~1e-8) + one 8x8 SPD LDL^T solve per token.

Per-chunk layouts (TC=4096 tokens, token_local = 32*p + q):
  A' (tokens on partitions): [128, (q=32, feat)]
  B  (features on partitions, tokens on free), via PE transpose:
    (d)-space  [64  = 8*q3+d,   (H=4, 128p)]    q = 8H + q3
    (j)-space  [128 = 16*q3+j,  (H=4, 128p)]
    (mn)-space [128 = 64*qs+mn, (P=16, 128p)]   q = 2P + qs
"""

import numpy as np

try:  # concourse ships with the container; ensure it's importable
    import concourse  # noqa: F401
except ImportError:  # pragma: no cover
    import sys as _sys
    for _p in ("/opt/trn_rl_repo", "/root/.axon_site/_ro/trn_rl_repo"):
        if _p not in _sys.path:
            _sys.path.insert(0, _p)

LAST_EXEC_TIME_NS = None
LAST_TRACE = None
D = 8
NCORES = 8
NTOK = 8192
TC = 4096
NCHUNK = NTOK // TC
NP = 128
NQ = TC // NP      # 32


def _build_consts(L, W1, b1, W2, b2, Wr1, br1, Wr2, br2):
    f = np.float32
    L, W1, b1, W2, b2 = (np.asarray(a, np.float64) for a in (L, W1, b1, W2, b2))
    Wr1, br1, Wr2, br2 = (np.asarray(a, np.float64) for a in (Wr1, br1, Wr2, br2))
    G0 = L @ L.T + 1e-4 * np.eye(D)
    W2r = W2.reshape(16, D, D)
    W2sym = (0.5 * (W2r + np.swapaxes(W2r, 1, 2))).reshape(16, 64)
    b2r = b2.reshape(D, D)
    b2sym = (0.5 * (b2r + b2r.T)).reshape(64)
    W2sym2 = (W2r + np.swapaxes(W2r, 1, 2)).reshape(16, 64)
    Wdr0 = Wr1 * Wr2[:, 0][None, :]          # [r, j] = Wr1[r,j]*Wr2[j,0]

    def blockdiag(w, g):
        kin, mout = w.shape
        out = np.zeros((g * kin, g * mout), dtype=np.float64)
        for i in range(g):
            out[i * kin:(i + 1) * kin, i * mout:(i + 1) * mout] = w
        return out

    C = {}
    C["eye128"] = np.eye(128)
    C["ones1"] = np.ones((1, 128))
    g0row = np.zeros((1, 128))
    g0row[0, :64] = (10.0 * G0).reshape(64)
    g0row[0, 64:] = (10.0 * G0).reshape(64)
    C["g0row10"] = g0row
    C["bd_w1"] = blockdiag(W1, 8)                 # [64,128]
    C["bd_wr1"] = blockdiag(Wr1, 8)               # [64,64]
    C["bd_g0"] = blockdiag(G0, 8)                 # [64,64]
    # per-Pl expanded (q3-selective) weights: [(q3,j),(qs,mn)] = d_{q3,2Pl+qs}*W
    for Pl in range(4):
        w = np.zeros((128, 128))
        w2 = np.zeros((128, 128))
        for qs in range(2):
            q3 = 2 * Pl + qs
            w[q3 * 16:(q3 + 1) * 16, qs * 64:(qs + 1) * 64] = W2sym
            w2[q3 * 16:(q3 + 1) * 16, qs * 64:(qs + 1) * 64] = W2sym2
        C[f"bd_w2sym_{Pl}"] = w
        C[f"bd_w2sym2_{Pl}"] = w2
    C["bd_w2q"] = blockdiag(0.1 * W2.T, 2)        # [128,32]
    sel = np.zeros((128, 16))
    for qs in range(2):
        for n in range(D):
            for r in range(D):
                sel[qs * 64 + n * D + r, qs * D + r] = 1.0
    C["selc"] = sel
    ones2 = np.zeros((128, 2))
    ones2[:64, 0] = 1.0
    ones2[64:, 1] = 1.0
    C["onesc"] = ones2
    ones8 = np.zeros((64, 8))
    for q3 in range(8):
        ones8[q3 * D:(q3 + 1) * D, q3] = 1.0
    C["ones8c"] = ones8
    C["w1tc"] = blockdiag(W1.T, 8)                # [128,64]
    C["wdr0c"] = blockdiag(Wdr0.T, 8)             # [64,64]
    wr2c = np.zeros((64, 8))
    for q3 in range(8):
        wr2c[q3 * D:(q3 + 1) * D, q3] = Wr2[:, 0]
    C["wr2c"] = wr2c
    # repXc_Pl [64,128]: [(q3,d),(qs,(n,r))] = d_{q3,2Pl+qs} * d_{d,n or d,r}
    for Pl in range(4):
        rep1 = np.zeros((64, 128))
        rep2 = np.zeros((64, 128))
        for qs in range(2):
            q3 = 2 * Pl + qs
            for d in range(D):
                for r in range(D):
                    rep1[q3 * D + d, qs * 64 + d * D + r] = 1.0   # n = d
                    rep2[q3 * D + d, qs * 64 + r * D + d] = 1.0   # r = d
        C[f"rep1c_{Pl}"] = rep1
        C[f"rep2c_{Pl}"] = rep2
    C["b1c"] = np.tile(b1, 8)                     # [128]
    C["br1c"] = np.tile(br1, 8)                   # [64]
    C["b2symc"] = np.tile(b2sym, 2)               # [128]
    C = {k: np.ascontiguousarray(v, dtype=f) for k, v in C.items()}
    return C, float(br2[0])


CONST_SHAPES = {
    "eye128": (128, 128), "ones1": (1, 128), "g0row10": (1, 128),
    "bd_w1": (64, 128), "bd_wr1": (64, 64), "bd_g0": (64, 64),
    "bd_w2q": (128, 32),
    "selc": (128, 16), "onesc": (128, 2), "ones8c": (64, 8),
    "w1tc": (128, 64), "wdr0c": (64, 64), "wr2c": (64, 8),
    "b1c": (128,), "br1c": (64,), "b2symc": (128,),
    **{f"bd_w2sym_{p}": (128, 128) for p in range(4)},
    **{f"bd_w2sym2_{p}": (128, 128) for p in range(4)},
    **{f"rep1c_{p}": (64, 128) for p in range(4)},
    **{f"rep2c_{p}": (64, 128) for p in range(4)},
}


def _emit(nc, tc, ctx, dram, br2f):
    import concourse.mybir as mybir

    f32 = mybir.dt.float32
    AF = mybir.ActivationFunctionType
    OP = mybir.AluOpType

    consts = ctx.enter_context(tc.tile_pool(name="consts", bufs=1))
    sb = ctx.enter_context(tc.tile_pool(name="sb", bufs=2))
    sbig = ctx.enter_context(tc.tile_pool(name="sbig", bufs=1))
    sbA = ctx.enter_context(tc.tile_pool(name="sbA", bufs=2))
    wps = ctx.enter_context(tc.tile_pool(name="wps", bufs=3, space="PSUM"))
    gtps = ctx.enter_context(tc.tile_pool(name="gtps", bufs=1, space="PSUM"))
    scps = ctx.enter_context(tc.tile_pool(name="scps", bufs=1, space="PSUM"))

    cs = {}
    for name, shape in CONST_SHAPES.items():
        if len(shape) == 1:
            t = consts.tile([shape[0], 1], f32, name=name, tag=name)
            nc.sync.dma_start(out=t[:, :],
                              in_=dram[name].rearrange("(p one) -> p one", one=1))
        else:
            t = consts.tile(list(shape), f32, name=name, tag=name)
            nc.sync.dma_start(out=t[:, :], in_=dram[name][:, :])
        cs[name] = t
    ident = cs["eye128"]
    br2t = consts.tile([128, 1], f32, name="br2t")
    nc.vector.memset(br2t[:, :], br2f)
    br2h = consts.tile([128, 1], f32, name="br2h")
    nc.vector.memset(br2h[:, :], 0.5 * br2f)
    onet = consts.tile([128, 1], f32, name="onet")
    nc.vector.memset(onet[:, :], 1.0)

    def dram_chunk(t, c):
        return t[c * TC:(c + 1) * TC, :].rearrange("(p q) d -> p (q d)", q=NQ)

    def transpose2(src, tag):
        """[128, 256] A'-(q,d) -> B (d)-space [64 = 8*q3+d, (H4, 128p)] in SBUF."""
        out = sb.tile([64, 512], f32, tag=tag)
        pt = wps.tile([128, 512], f32, tag="work_ps")
        for H in range(4):
            nc.tensor.matmul(pt[:64, H * 128:(H + 1) * 128],
                             src[:, H * 64:(H + 1) * 64],
                             ident[:, :], is_transpose=True, start=True, stop=True)
        nc.vector.tensor_copy(out[:, :], pt[:64, :])
        return out

    def emit_call(xT, vT, vA, aA):
        """One christoffel+contraction; writes acceleration into aA [128,(q,8)]."""

        # ---------- forward matmuls ((j)/(d)-space) ----------
        u_ps = wps.tile([128, 512], f32, tag="work_ps")
        s_ps = wps.tile([128, 512], f32, tag="work_ps")
        for H in range(4):
            rhs = xT[:, H * 128:(H + 1) * 128]
            sl = slice(H * 128, (H + 1) * 128)
            nc.tensor.matmul(u_ps[:, sl], cs["bd_w1"][:, :], rhs, start=True, stop=True)
            nc.tensor.matmul(s_ps[:64, sl], cs["bd_wr1"][:, :], rhs, start=True, stop=True)
        a1B = sb.tile([128, 512], f32, tag="a1B")
        gpuB = sb.tile([128, 512], f32, tag="gpuB")
        nc.scalar.activation(a1B[:, :], u_ps[:, :], AF.Gelu, bias=cs["b1c"][:, :])
        nc.scalar.activation(gpuB[:, :], u_ps[:, :], AF.Derivative_Gelu,
                             bias=cs["b1c"][:, :])
        a2B = sb.tile([64, 512], f32, tag="a2B")
        gpsB = sb.tile([64, 512], f32, tag="gpsB")
        nc.scalar.activation(a2B[:, :], s_ps[:64, :], AF.Gelu, bias=cs["br1c"][:, :])
        nc.scalar.activation(gpsB[:, :], s_ps[:64, :], AF.Derivative_Gelu,
                             bias=cs["br1c"][:, :])

        c_ps = wps.tile([128, 512], f32, tag="work_ps")
        gv_ps = wps.tile([128, 512], f32, tag="work_ps")
        for H in range(4):
            rhv = vT[:, H * 128:(H + 1) * 128]
            sl = slice(H * 128, (H + 1) * 128)
            nc.tensor.matmul(c_ps[:, sl], cs["bd_w1"][:, :], rhv, start=True, stop=True)
            nc.tensor.matmul(gv_ps[:64, sl], cs["bd_g0"][:, :], rhv, start=True, stop=True)
        cgB = sb.tile([128, 512], f32, tag="cgB")
        nc.vector.tensor_mul(cgB[:, :], c_ps[:, :], gpuB[:, :])
        m1B = sb.tile([64, 512], f32, tag="m1B")
        nc.vector.tensor_mul(m1B[:, :], gv_ps[:64, :], vT[:, :])

        # ---------- scalar-channel + small A'-folds (PSUM pack) ----------
        # pack: [0:32) t | [32:64) QG | [64:96) QE | [96:128) unused
        # [128:384) dr0 | [384:640) T1E | [640:896) T2E
        pk = scps.tile([128, 1024], f32, tag="pack_ps")
        t_ps = pk[:, 0:32]
        qg_ps = pk[:, 32:64]
        qe_ps = pk[:, 64:96]
        dr0_ps = pk[:, 128:384]
        t1e_ps = pk[:, 384:640]
        t2e_ps = pk[:, 640:896]
        for H in range(4):
            hsl = slice(H * 128, (H + 1) * 128)
            nc.tensor.matmul(t_ps[:, H * 8:(H + 1) * 8], a2B[:, hsl],
                             cs["wr2c"][:, :], start=True, stop=True)
            nc.tensor.matmul(qg_ps[:, H * 8:(H + 1) * 8], m1B[:, hsl],
                             cs["ones8c"][:, :], start=True, stop=True)
            nc.tensor.matmul(dr0_ps[:, H * 64:(H + 1) * 64], gpsB[:, hsl],
                             cs["wdr0c"][:, :], start=True, stop=True)

        def stile(tag):
            return sbA.tile([128, 32], f32, tag=tag, name=tag)
        rrawA, sigA, rA, rinvA, kapA, tmpA, uA, absA = (
            stile(t) for t in ["rrawA", "sigA", "rA", "rinvA", "kapA", "tmpA",
                               "uA", "absA"])
        # u = t + br2; softplus(u) = ln(exp(-|u|) + 1) + relu(u)
        nc.scalar.activation(uA[:, :], t_ps[:, :], AF.Identity, bias=br2t[:, :])
        nc.scalar.activation(absA[:, :], t_ps[:, :], AF.Abs, bias=br2t[:, :])
        nc.scalar.activation(absA[:, :], absA[:, :], AF.Exp, scale=-1.0)
        nc.scalar.activation(absA[:, :], absA[:, :], AF.Ln, bias=onet[:, :])
        nc.vector.tensor_scalar_max(rrawA[:, :], uA[:, :], 0.0)
        nc.vector.tensor_add(rrawA[:, :], rrawA[:, :], absA[:, :])
        # sigmoid(u) = 0.5 + 0.5*tanh(u/2)
        nc.scalar.activation(sigA[:, :], t_ps[:, :], AF.Tanh, scale=0.5,
                             bias=br2h[:, :])
        nc.vector.tensor_scalar(out=sigA[:, :], in0=sigA[:, :], scalar1=0.5,
                                scalar2=0.5, op0=OP.mult, op1=OP.add)
        nc.vector.tensor_scalar_max(rA[:, :], rrawA[:, :], 0.1)
        nc.vector.tensor_scalar_min(rA[:, :], rA[:, :], 10.0)
        nc.vector.reciprocal(rinvA[:, :], rA[:, :])
        nc.vector.tensor_scalar(out=kapA[:, :], in0=rrawA[:, :], scalar1=0.1,
                                scalar2=None, op0=OP.is_gt)
        nc.vector.tensor_scalar(out=tmpA[:, :], in0=rrawA[:, :], scalar1=10.0,
                                scalar2=None, op0=OP.is_lt)
        nc.vector.tensor_mul(kapA[:, :], kapA[:, :], tmpA[:, :])
        nc.vector.tensor_mul(kapA[:, :], kapA[:, :], sigA[:, :])

        # ---------- (mn)-space stream ----------
        tanhSB = sbig.tile([128, 2048], f32, tag="tanhSB")
        tanhpB = sbig.tile([128, 2048], f32, tag="tanhpB")
        wtB = sbig.tile([128, 2048], f32, tag="wtB")
        vr1B = sbig.tile([128, 2048], f32, tag="vr1B")
        vvTB = sbig.tile([128, 2048], f32, tag="vvTB")
        ppB = sbig.tile([128, 2048], f32, tag="ppB")
        qqB = sbig.tile([128, 2048], f32, tag="qqB")
        t1preB = sbig.tile([128, 2048], f32, tag="t1preB")
        gA = sbig.tile([128, 2048], f32, tag="gA")
        invdA = sbA.tile([128, 256], f32, tag="invdA")
        wcolA = sbA.tile([128, 224], f32, tag="wcolA")      # (q32, 7)
        tscrA = sbA.tile([128, 1568], f32, tag="tscrA")     # (q32, 49)
        qa_ps = wps.tile([64, 512], f32, tag="qa_ps", bufs=1)
        qb_ps = wps.tile([64, 512], f32, tag="qb_ps", bufs=1)

        for H in range(4):
            hsl = slice(H * 512, (H + 1) * 512)
            S_ps = wps.tile([128, 512], f32, tag="work_ps")
            bs_ps = wps.tile([128, 512], f32, tag="work_ps")
            v1_ps = wps.tile([128, 512], f32, tag="work_ps")
            v2_ps = wps.tile([128, 512], f32, tag="work_ps")
            hb = slice(H * 128, (H + 1) * 128)
            for Pl in range(4):
                psl = slice(Pl * 128, (Pl + 1) * 128)
                nc.tensor.matmul(S_ps[:, psl], cs[f"bd_w2sym_{Pl}"][:, :],
                                 a1B[:, hb], start=True, stop=True)
                nc.tensor.matmul(bs_ps[:, psl], cs[f"bd_w2sym2_{Pl}"][:, :],
                                 cgB[:, hb], start=True, stop=True)
                nc.tensor.matmul(v1_ps[:, psl], cs[f"rep1c_{Pl}"][:, :],
                                 vT[:, hb], start=True, stop=True)
                nc.tensor.matmul(v2_ps[:, psl], cs[f"rep2c_{Pl}"][:, :],
                                 vT[:, hb], start=True, stop=True)
            nc.scalar.activation(tanhSB[:, hsl], S_ps[:, :], AF.Tanh,
                                 bias=cs["b2symc"][:, :])
            nc.scalar.activation(tanhpB[:, hsl], tanhSB[:, hsl], AF.Square)
            nc.scalar.activation(tanhpB[:, hsl], tanhpB[:, hsl], AF.Identity,
                                 scale=-1.0, bias=onet[:, :])
            nc.vector.tensor_copy(vr1B[:, hsl], v1_ps[:, :])
            nc.vector.tensor_mul(vvTB[:, hsl], vr1B[:, hsl], v2_ps[:, :])
            nc.vector.tensor_mul(wtB[:, hsl], tanhpB[:, hsl], bs_ps[:, :])
            nc.vector.tensor_mul(t1preB[:, hsl], wtB[:, hsl], vr1B[:, hsl])
            nc.gpsimd.tensor_mul(ppB[:, hsl], tanhpB[:, hsl], vvTB[:, hsl])
            nc.gpsimd.tensor_mul(qqB[:, hsl], tanhSB[:, hsl], vvTB[:, hsl])

            for Pl in range(4):
                P = 4 * H + Pl
                psl128 = slice(P * 128, (P + 1) * 128)
                qdst = (qa_ps if Pl < 2 else qb_ps)
                nc.tensor.matmul(
                    qdst[32 * (Pl % 2):32 * (Pl % 2) + 32, H * 128:(H + 1) * 128],
                    cs["bd_w2q"][:, :], ppB[:, psl128], start=True, stop=True)
                nc.tensor.matmul(t1e_ps[:, P * 16:(P + 1) * 16],
                                 t1preB[:, psl128], cs["selc"][:, :],
                                 start=True, stop=True)
                nc.tensor.matmul(qe_ps[:, P * 2:(P + 1) * 2],
                                 qqB[:, psl128], cs["onesc"][:, :],
                                 start=True, stop=True)

            # g-tilde for this H: psum [128, (Pl4, qs2, mn64)]
            gt_ps = gtps.tile([128, 512], f32, tag="gt_ps")
            for Pl in range(4):
                P = 4 * H + Pl
                gsl = slice(Pl * 128, (Pl + 1) * 128)
                nc.tensor.matmul(gt_ps[:, gsl], cs["ones1"][:1, :],
                                 cs["g0row10"][:1, :], start=True, stop=False)
                nc.tensor.matmul(gt_ps[:, gsl], tanhSB[:, P * 128:(P + 1) * 128],
                                 ident[:, :], is_transpose=True,
                                 start=False, stop=True)
            # LDL k=0 on this H (src = gt_ps), writes gA records for q in H-range
            q0 = 8 * H                      # first q of this H
            # invd0: diag mn=0
            nc.vector.reciprocal(
                invdA[:, q0:q0 + 8],
                gt_ps[:, :].rearrange("p (q mn) -> p q mn", mn=64)[:, :, 0])
            # wcol0: col0 rows 1..7 -> wcolA[(q in H), 7]
            wv = wcolA[:, 7 * q0:7 * (q0 + 8)].rearrange("p (q i) -> p q i", i=7)
            gtv = gt_ps[:, :].rearrange("p (q i j) -> p q i j", i=8, j=8)
            nc.vector.tensor_copy(wv[:, :, :], gtv[:, :, 1:8, 0])
            # l0 = wcol0 * invd0 -> gA col0
            gAv = gA[:, :].rearrange("p (q i j) -> p q i j", i=8, j=8)
            nc.vector.tensor_tensor(
                out=gAv[:, q0:q0 + 8, 1:8, 0], in0=wv[:, :, :],
                in1=invdA[:, q0:q0 + 8, None].broadcast_to([128, 8, 7]),
                op=OP.mult)
            # outer0 = l0_i * w0_j
            tv = tscrA[:, 49 * q0:49 * (q0 + 8)].rearrange(
                "p (q i j) -> p q i j", i=7, j=7)
            nc.vector.tensor_tensor(
                out=tv[:, :, :, :],
                in0=gAv[:, q0:q0 + 8, 1:8, 0:1].broadcast_to([128, 8, 7, 7]),
                in1=wv[:, :, None, :].broadcast_to([128, 8, 7, 7]),
                op=OP.mult)
            # sub0: gA rect rows1..7 = gt - outer
            nc.vector.tensor_tensor(
                out=gAv[:, q0:q0 + 8, 1:8, 1:8], in0=gtv[:, :, 1:8, 1:8],
                in1=tv[:, :, :, :], op=OP.subtract)

        # ---------- LDL k=1..7 on gA (all 32 q at once) ----------
        gAv = gA[:, :].rearrange("p (q i j) -> p q i j", i=8, j=8)
        wv7 = wcolA[:, :].rearrange("p (q i) -> p q i", i=7)
        tv7 = tscrA[:, :].rearrange("p (q i j) -> p q i j", i=7, j=7)
        for k in range(1, 7):
            m = 7 - k
            nc.vector.reciprocal(invdA[:, 32 * k:32 * (k + 1)], gAv[:, :, k, k])
            nc.vector.tensor_copy(wv7[:, :, :m], gAv[:, :, k + 1:8, k])
            nc.vector.tensor_tensor(
                out=gAv[:, :, k + 1:8, k], in0=wv7[:, :, :m],
                in1=invdA[:, 32 * k:32 * (k + 1), None].broadcast_to([128, 32, m]),
                op=OP.mult)
            nc.vector.tensor_tensor(
                out=tv7[:, :, :m, :m],
                in0=gAv[:, :, k + 1:8, k:k + 1].broadcast_to([128, 32, m, m]),
                in1=wv7[:, :, None, :m].broadcast_to([128, 32, m, m]),
                op=OP.mult)
            nc.vector.tensor_tensor(
                out=gAv[:, :, k + 1:8, k + 1:8], in0=gAv[:, :, k + 1:8, k + 1:8],
                in1=tv7[:, :, :m, :m], op=OP.subtract)
        nc.vector.reciprocal(invdA[:, 224:256], gAv[:, :, 7, 7])

        # ---------- q -> gpq -> T2E ----------
        gpqB = sb.tile([128, 512], f32, tag="gpqB")
        nc.vector.tensor_mul(gpqB[:64, :], gpuB[:64, :], qa_ps[:, :])
        nc.vector.tensor_mul(gpqB[64:, :], gpuB[64:, :], qb_ps[:, :])
        for H in range(4):
            nc.tensor.matmul(t2e_ps[:, H * 64:(H + 1) * 64],
                             gpqB[:, H * 128:(H + 1) * 128], cs["w1tc"][:, :],
                             start=True, stop=True)

        # ---------- Q, coefZ, z ----------
        qgA, qaA, czA, caA, dvA = (stile(t) for t in
                                   ["qgA", "qaA", "czA", "caA", "dvA"])
        nc.vector.tensor_copy(qgA[:, :], qg_ps[:, :])
        nc.vector.scalar_tensor_tensor(out=qaA[:, :], in0=qe_ps[:, :], scalar=0.1,
                                       in1=qgA[:, :], op0=OP.mult, op1=OP.add)
        nc.vector.tensor_mul(czA[:, :], qaA[:, :], kapA[:, :])
        nc.vector.tensor_mul(czA[:, :], czA[:, :], rinvA[:, :])
        # dv = sum_r dr0*v
        dvmA = sbA.tile([128, 256], f32, tag="dvmA")
        nc.vector.tensor_mul(dvmA[:, :], dr0_ps[:, :], vA[:, :])
        nc.vector.tensor_reduce(
            dvA[:, :], dvmA[:, :].rearrange("p (q r) -> p q r", r=8),
            axis=mybir.AxisListType.X, op=OP.add)
        nc.vector.scalar_tensor_tensor(out=caA[:, :], in0=kapA[:, :], scalar=2.0,
                                       in1=dvA[:, :], op0=OP.mult, op1=OP.mult)
        nc.vector.tensor_mul(caA[:, :], caA[:, :], rinvA[:, :])
        # z = 0.05*T1E - 0.5*T2E - cz*dr0
        t1s = sbA.tile([128, 256], f32, tag="t1s")
        zA = sbA.tile([128, 256], f32, tag="zA")
        nc.vector.tensor_tensor(
            out=t1s[:, :].rearrange("p (q r) -> p q r", r=8),
            in0=dr0_ps.rearrange("p (q r) -> p q r", r=8),
            in1=czA[:, :, None].broadcast_to([128, 32, 8]),
            op=OP.mult)
        nc.vector.scalar_tensor_tensor(out=zA[:, :], in0=t2e_ps[:, :], scalar=-0.5,
                                       in1=t1s[:, :], op0=OP.mult, op1=OP.subtract)
        nc.vector.scalar_tensor_tensor(out=zA[:, :], in0=t1e_ps[:, :], scalar=0.05,
                                       in1=zA[:, :], op0=OP.mult, op1=OP.add)

        # ---------- solve gA y = z ----------
        yv = zA[:, :].rearrange("p (q r) -> p q r", r=8)        # in-place y
        sv = sbA.tile([128, 224], f32, tag="solve_scr")
        svv = sv[:, :].rearrange("p (q i) -> p q i", i=7)
        for k in range(0, 7):
            m = 7 - k
            nc.vector.tensor_tensor(
                out=svv[:, :, :m], in0=gAv[:, :, k + 1:8, k],
                in1=yv[:, :, k:k + 1].broadcast_to([128, 32, m]), op=OP.mult)
            nc.vector.tensor_tensor(
                out=yv[:, :, k + 1:8], in0=yv[:, :, k + 1:8],
                in1=svv[:, :, :m], op=OP.subtract)
        nc.vector.tensor_tensor(
            out=yv[:, :, :],
            in0=yv[:, :, :],
            in1=invdA[:, :].rearrange("p (k q) -> p q k", q=32),
            op=OP.mult)
        for k in range(7, 0, -1):
            nc.vector.tensor_tensor(
                out=svv[:, :, :k], in0=gAv[:, :, k, 0:k],
                in1=yv[:, :, k:k + 1].broadcast_to([128, 32, k]), op=OP.mult)
            nc.vector.tensor_tensor(
                out=yv[:, :, 0:k], in0=yv[:, :, 0:k],
                in1=svv[:, :, :k], op=OP.subtract)

        # ---------- a = -coefA*v - 10*y ----------
        t3 = sbA.tile([128, 256], f32, tag="t3")
        nc.vector.tensor_tensor(
            out=t3[:, :].rearrange("p (q r) -> p q r", r=8),
            in0=vA[:, :].rearrange("p (q r) -> p q r", r=8),
            in1=caA[:, :, None].broadcast_to([128, 32, 8]),
            op=OP.mult)
        nc.vector.scalar_tensor_tensor(out=aA[:, :], in0=zA[:, :], scalar=-10.0,
                                       in1=t3[:, :], op0=OP.mult, op1=OP.subtract)

    # ================= chunk loop =================
    for c in range(NCHUNK):
        xA = sbA.tile([128, 256], f32, tag="xA")
        vA = sbA.tile([128, 256], f32, tag="vA")
        nc.sync.dma_start(out=xA[:, :], in_=dram_chunk(dram["x"], c))
        nc.sync.dma_start(out=vA[:, :], in_=dram_chunk(dram["v"], c))
        xT = transpose2(xA, "xT")
        vT = transpose2(vA, "vT")

        aA1 = sbA.tile([128, 256], f32, tag="aA1")
        emit_call(xT, vT, vA, aA1)

        vmidA = sbA.tile([128, 256], f32, tag="vmidA")
        nc.vector.scalar_tensor_tensor(out=vmidA[:, :], in0=aA1[:, :], scalar=0.05,
                                       in1=vA[:, :], op0=OP.mult, op1=OP.add)
        xnewA = sbA.tile([128, 256], f32, tag="xnewA")
        nc.vector.scalar_tensor_tensor(out=xnewA[:, :], in0=vmidA[:, :], scalar=0.1,
                                       in1=xA[:, :], op0=OP.mult, op1=OP.add)
        nc.sync.dma_start(out=dram_chunk(dram["x_new"], c), in_=xnewA[:, :])

        xmidT = sb.tile([64, 512], f32, tag="xmidT")
        nc.vector.scalar_tensor_tensor(out=xmidT[:, :], in0=vT[:, :], scalar=0.05,
                                       in1=xT[:, :], op0=OP.mult, op1=OP.add)
        vmidT = transpose2(vmidA, "vmidT")

        aA2 = sbA.tile([128, 256], f32, tag="aA2")
        emit_call(xmidT, vmidT, vmidA, aA2)

        vnewA = sbA.tile([128, 256], f32, tag="vnewA")
        nc.vector.scalar_tensor_tensor(out=vnewA[:, :], in0=aA2[:, :], scalar=0.1,
                                       in1=vA[:, :], op0=OP.mult, op1=OP.add)
        nc.sync.dma_start(out=dram_chunk(dram["v_new"], c), in_=vnewA[:, :])


def _build_module(consts, br2f):
    import concourse.bacc as bacc
    import concourse.mybir as mybir
    import concourse.tile as tile
    from contextlib import ExitStack

    f32 = mybir.dt.float32
    nc = bacc.Bacc("TRN2", target_bir_lowering=False, debug=False,
                   num_devices=NCORES)
    dram = {}
    dram["x"] = nc.dram_tensor("x", [NTOK, D], f32, kind="ExternalInput").ap()
    dram["v"] = nc.dram_tensor("v", [NTOK, D], f32, kind="ExternalInput").ap()
    for name, arr in consts.items():
        dram[name] = nc.dram_tensor(name, list(arr.shape), f32,
                                    kind="ExternalInput").ap()
    dram["x_new"] = nc.dram_tensor("x_new", [NTOK, D], f32,
                                   kind="ExternalOutput").ap()
    dram["v_new"] = nc.dram_tensor("v_new", [NTOK, D], f32,
                                   kind="ExternalOutput").ap()
    with tile.TileContext(nc) as tc:
        with ExitStack() as ctx:
            _emit(nc, tc, ctx, dram, br2f)
    nc.compile()
    return nc


def kernel(x, v, L, W1, b1, W2, b2, Wr1, br1, Wr2, br2):
    x = np.ascontiguousarray(np.asarray(x, dtype=np.float32))
    v = np.ascontiguousarray(np.asarray(v, dtype=np.float32))
    consts, br2f = _build_consts(L, W1, b1, W2, b2, Wr1, br1, Wr2, br2)
    nc = _build_module(consts, br2f)

    from concourse.bass_utils import run_bass_kernel_spmd
    in_maps = []
    for c in range(NCORES):
        m = {"x": np.ascontiguousarray(x[c]), "v": np.ascontiguousarray(v[c])}
        m.update(consts)
        in_maps.append(m)
    import os as _os
    trace = _os.environ.get("KERNEL_TRACE", "0") == "1"
    tmpdir = _os.environ.get("KERNEL_TRACE_DIR") or None
    res = run_bass_kernel_spmd(nc, in_maps, core_ids=list(range(NCORES)),
                               trace=trace, tmpdir=tmpdir)
    global LAST_EXEC_TIME_NS, LAST_TRACE
    LAST_EXEC_TIME_NS = res.exec_time_ns
    LAST_TRACE = res.instructions_and_trace
    x_new = np.stack([r["x_new"] for r in res.results]).astype(np.float32)
    v_new = np.stack([r["v_new"] for r in res.results]).astype(np.float32)
    return (x_new, v_new)



# revision 7
# speedup vs baseline: 1.9161x; 1.9161x over previous
"""Trainium2 Bass kernel for nn_CognitiveManifold (geodesic RK2 step).

8 NeuronCores, pure data parallel: 8192 tokens/core, full inputs in, full
outputs out. Analytic metric derivatives + one 8x8 SPD LDL^T solve per
token. v2: bf16 tensor-engine path (fp32 kept for the clip-sensitive
R-channel and the LDL/solve), Pl-major 512-wide matmuls.

Per-chunk layouts (TC=4096 tokens, token_local = 32*p + q):
  A (tokens on partitions): [128, (q=32, feat)] fp32
  B (features on partitions, tokens on free), via PE transpose:
    (d)-space  [64  = 8*q3+d,   (H=4, 128p)]    q = 8H + q3
    (j)-space  [128 = 16*q3+j,  (H=4, 128p)]
    (mn)-space [128 = 64*qs+mn, (Pl4|H4|p128)]  q = 8H + 2Pl + qs
"""

import numpy as np
from ml_dtypes import bfloat16

try:  # concourse ships with the container; ensure it's importable
    import concourse  # noqa: F401
except ImportError:  # pragma: no cover
    import sys as _sys
    for _p in ("/opt/trn_rl_repo", "/root/.axon_site/_ro/trn_rl_repo"):
        if _p not in _sys.path:
            _sys.path.insert(0, _p)

LAST_EXEC_TIME_NS = None
LAST_TRACE = None
D = 8
NCORES = 8
NTOK = 8192
TC = 4096
NCHUNK = NTOK // TC
NP = 128
NQ = TC // NP      # 32


def _build_consts(L, W1, b1, W2, b2, Wr1, br1, Wr2, br2):
    L, W1, b1, W2, b2 = (np.asarray(a, np.float64) for a in (L, W1, b1, W2, b2))
    Wr1, br1, Wr2, br2 = (np.asarray(a, np.float64) for a in (Wr1, br1, Wr2, br2))
    G0 = L @ L.T + 1e-4 * np.eye(D)
    W2r = W2.reshape(16, D, D)
    W2sym = (0.5 * (W2r + np.swapaxes(W2r, 1, 2))).reshape(16, 64)
    b2r = b2.reshape(D, D)
    b2sym = (0.5 * (b2r + b2r.T)).reshape(64)
    W2sym2 = (W2r + np.swapaxes(W2r, 1, 2)).reshape(16, 64)
    Wdr0 = Wr1 * Wr2[:, 0][None, :]          # [r, j] = Wr1[r,j]*Wr2[j,0]

    def blockdiag(w, g):
        kin, mout = w.shape
        out = np.zeros((g * kin, g * mout), dtype=np.float64)
        for i in range(g):
            out[i * kin:(i + 1) * kin, i * mout:(i + 1) * mout] = w
        return out

    B = {}   # bf16 consts
    F = {}   # fp32 consts
    B["eye128b"] = np.eye(128)
    F["eye128"] = np.eye(128)
    B["bd_w1"] = blockdiag(W1, 8)                 # [64,128]
    F["bd_wr1"] = blockdiag(Wr1, 8)               # [64,64] fp32 (R-channel)
    B["bd_g0"] = blockdiag(G0, 8)                 # [64,64]
    # per-Pl expanded (q3-selective) weights: [(q3,j),(qs,mn)] = d_{q3,2Pl+qs}*W
    for Pl in range(4):
        w = np.zeros((128, 128))
        w2 = np.zeros((128, 128))
        for qs in range(2):
            q3 = 2 * Pl + qs
            w[q3 * 16:(q3 + 1) * 16, qs * 64:(qs + 1) * 64] = W2sym
            w2[q3 * 16:(q3 + 1) * 16, qs * 64:(qs + 1) * 64] = W2sym2
        B[f"bd_w2sym_{Pl}"] = w
        B[f"bd_w2sym2_{Pl}"] = w2
    B["bd_w2q"] = blockdiag(0.1 * W2.T, 2)        # [128,32]
    sel = np.zeros((128, 16))
    for qs in range(2):
        for n in range(D):
            for r in range(D):
                sel[qs * 64 + n * D + r, qs * D + r] = 1.0
    B["selc"] = sel
    ones2 = np.zeros((128, 2))
    ones2[:64, 0] = 1.0
    ones2[64:, 1] = 1.0
    B["onesc"] = ones2
    ones8 = np.zeros((64, 8))
    for q3 in range(8):
        ones8[q3 * D:(q3 + 1) * D, q3] = 1.0
    B["ones8c"] = ones8
    B["w1tc"] = blockdiag(W1.T, 8)                # [128,64]
    B["wdr0c"] = blockdiag(Wdr0.T, 8)             # [64,64]
    wr2c = np.zeros((64, 8))
    for q3 in range(8):
        wr2c[q3 * D:(q3 + 1) * D, q3] = Wr2[:, 0]
    F["wr2c"] = wr2c                              # fp32 (R-channel)
    # repXc_Pl [64,128]: [(q3,d),(qs,(n,r))] = d_{q3,2Pl+qs} * d_{d,n or d,r}
    for Pl in range(4):
        rep1 = np.zeros((64, 128))
        rep2 = np.zeros((64, 128))
        for qs in range(2):
            q3 = 2 * Pl + qs
            for d in range(D):
                for r in range(D):
                    rep1[q3 * D + d, qs * 64 + d * D + r] = 1.0   # n = d
                    rep2[q3 * D + d, qs * 64 + r * D + d] = 1.0   # r = d
        B[f"rep1c_{Pl}"] = rep1
        B[f"rep2c_{Pl}"] = rep2
    F["b1c"] = np.tile(b1, 8)                     # [128]
    F["br1c"] = np.tile(br1, 8)                   # [64]
    F["b2symc"] = np.tile(b2sym, 2)               # [128]
    F["g0colB"] = np.tile((10.0 * G0).reshape(64), 2)  # [128]
    consts = {k: np.ascontiguousarray(v, dtype=bfloat16) for k, v in B.items()}
    consts.update({k: np.ascontiguousarray(v, dtype=np.float32)
                   for k, v in F.items()})
    return consts, float(br2[0])


CONST_DTYPES = dict(
    **{k: "bf16" for k in
       ["eye128b", "bd_w1", "bd_g0", "bd_w2q", "selc", "onesc", "ones8c",
        "w1tc", "wdr0c"]
       + [f"bd_w2sym_{p}" for p in range(4)]
       + [f"bd_w2sym2_{p}" for p in range(4)]
       + [f"rep1c_{p}" for p in range(4)]
       + [f"rep2c_{p}" for p in range(4)]},
    **{k: "f32" for k in
       ["eye128", "bd_wr1", "wr2c", "b1c", "br1c", "b2symc", "g0colB"]},
)

CONST_SHAPES = {
    "eye128b": (128, 128), "eye128": (128, 128),
    "bd_w1": (64, 128), "bd_wr1": (64, 64), "bd_g0": (64, 64),
    "bd_w2q": (128, 32),
    "selc": (128, 16), "onesc": (128, 2), "ones8c": (64, 8),
    "w1tc": (128, 64), "wdr0c": (64, 64), "wr2c": (64, 8),
    "b1c": (128,), "br1c": (64,), "b2symc": (128,), "g0colB": (128,),
    **{f"bd_w2sym_{p}": (128, 128) for p in range(4)},
    **{f"bd_w2sym2_{p}": (128, 128) for p in range(4)},
    **{f"rep1c_{p}": (64, 128) for p in range(4)},
    **{f"rep2c_{p}": (64, 128) for p in range(4)},
}


def _emit(nc, tc, ctx, dram, br2f):
    import concourse.mybir as mybir

    f32 = mybir.dt.float32
    bf = mybir.dt.bfloat16
    AF = mybir.ActivationFunctionType
    OP = mybir.AluOpType

    consts = ctx.enter_context(tc.tile_pool(name="consts", bufs=1))
    sbB = ctx.enter_context(tc.tile_pool(name="sbB", bufs=2))    # big bf16 B
    sbP = ctx.enter_context(tc.tile_pool(name="sbP", bufs=2))    # per-Pl bf16
    sbF = ctx.enter_context(tc.tile_pool(name="sbF", bufs=2))    # fwd B tiles
    sbA = ctx.enter_context(tc.tile_pool(name="sbA", bufs=2))    # A-layout f32
    wps = ctx.enter_context(tc.tile_pool(name="wps", bufs=2, space="PSUM"))
    v12 = ctx.enter_context(tc.tile_pool(name="v12", bufs=1, space="PSUM"))
    qgt = ctx.enter_context(tc.tile_pool(name="qgt", bufs=1, space="PSUM"))
    scps = ctx.enter_context(tc.tile_pool(name="scps", bufs=1, space="PSUM"))

    cs = {}
    for name, shape in CONST_SHAPES.items():
        dt = bf if CONST_DTYPES[name] == "bf16" else f32
        if len(shape) == 1:
            t = consts.tile([shape[0], 1], dt, name=name, tag=name)
            nc.sync.dma_start(out=t[:, :],
                              in_=dram[name].rearrange("(p one) -> p one", one=1))
        else:
            t = consts.tile(list(shape), dt, name=name, tag=name)
            nc.sync.dma_start(out=t[:, :], in_=dram[name][:, :])
        cs[name] = t
    identb = cs["eye128b"]
    ident32 = cs["eye128"]
    br2t = consts.tile([128, 1], f32, name="br2t")
    nc.vector.memset(br2t[:, :], br2f)
    br2h = consts.tile([128, 1], f32, name="br2h")
    nc.vector.memset(br2h[:, :], 0.5 * br2f)
    onet = consts.tile([128, 1], f32, name="onet")
    nc.vector.memset(onet[:, :], 1.0)

    def dram_chunk(t, c):
        return t[c * TC:(c + 1) * TC, :].rearrange("(p q) d -> p (q d)", q=NQ)

    def transpose32(src, tag):
        """[128, 256] A-(q,d) fp32 -> psum [64, (H4,128p)] fp32; returns the
        psum tile (caller copies out)."""
        pt = wps.tile([128, 512], f32, tag="work")
        for H in range(4):
            nc.tensor.matmul(pt[:64, H * 128:(H + 1) * 128],
                             src[:, H * 64:(H + 1) * 64],
                             ident32[:, :], is_transpose=True,
                             start=True, stop=True)
        return pt

    def transpose_bf(src, tag):
        """[128, 256] A-(q,d) bf16 -> [64, (H4,128p)] bf16 SBUF."""
        out = sbF.tile([64, 512], bf, tag=tag)
        pt = qgt.tile([128, 512], bf, tag="gt")
        for H in range(4):
            nc.tensor.matmul(pt[:64, H * 128:(H + 1) * 128],
                             src[:, H * 64:(H + 1) * 64],
                             identb[:, :], is_transpose=True,
                             start=True, stop=True)
        nc.vector.tensor_copy(out[:, :], pt[:64, :])
        return out

    def emit_call(xT32, xTb, vTb, vA, aA):
        """One christoffel+contraction; writes acceleration into aA [128,(q,8)]."""

        # ---------- Phase A: forward matmuls + activations ----------
        u_ps = wps.tile([128, 512], f32, tag="work")
        nc.tensor.matmul(u_ps[:, :], cs["bd_w1"][:, :], xTb[:, :],
                         start=True, stop=True)
        a1B = sbF.tile([128, 512], bf, tag="a1B")
        gpuB = sbF.tile([128, 512], bf, tag="gpuB")
        nc.scalar.activation(a1B[:, :], u_ps[:, :], AF.Gelu, bias=cs["b1c"][:, :])
        nc.scalar.activation(gpuB[:, :], u_ps[:, :], AF.Derivative_Gelu,
                             bias=cs["b1c"][:, :])

        s_ps = wps.tile([128, 512], f32, tag="work")
        nc.tensor.matmul(s_ps[:64, :], cs["bd_wr1"][:, :], xT32[:, :],
                         start=True, stop=True)
        a2B = sbF.tile([64, 512], f32, tag="a2B")
        gpsB = sbF.tile([64, 512], bf, tag="gpsB")
        nc.scalar.activation(a2B[:, :], s_ps[:64, :], AF.Gelu,
                             bias=cs["br1c"][:, :])
        nc.scalar.activation(gpsB[:, :], s_ps[:64, :], AF.Derivative_Gelu,
                             bias=cs["br1c"][:, :])

        c_ps = wps.tile([128, 512], f32, tag="work")
        nc.tensor.matmul(c_ps[:, :], cs["bd_w1"][:, :], vTb[:, :],
                         start=True, stop=True)
        cgB = sbF.tile([128, 512], bf, tag="cgB")
        nc.vector.tensor_tensor(out=cgB[:, :], in0=c_ps[:, :], in1=gpuB[:, :],
                                op=OP.mult)
        gv_ps = wps.tile([128, 512], f32, tag="work")
        nc.tensor.matmul(gv_ps[:64, :], cs["bd_g0"][:, :], vTb[:, :],
                         start=True, stop=True)
        m1B = sbF.tile([64, 512], bf, tag="m1B")
        nc.vector.tensor_tensor(out=m1B[:, :], in0=gv_ps[:64, :], in1=vTb[:, :],
                                op=OP.mult)

        # ---------- scalar-channel + small packs (PSUM pack) ----------
        # pack: [0:32) t | [32:64) QG | [64:96) QE | [96:128) unused
        # [128:384) dr0 | [384:640) T1E | [640:896) T2E
        pk = scps.tile([128, 1024], f32, tag="pack")
        t_ps = pk[:, 0:32]
        qg_ps = pk[:, 32:64]
        qe_ps = pk[:, 64:96]
        dr0_ps = pk[:, 128:384]
        t1e_ps = pk[:, 384:640]
        t2e_ps = pk[:, 640:896]
        for H in range(4):
            hsl = slice(H * 128, (H + 1) * 128)
            nc.tensor.matmul(t_ps[:, H * 8:(H + 1) * 8], a2B[:, hsl],
                             cs["wr2c"][:, :], start=True, stop=True)
            nc.tensor.matmul(qg_ps[:, H * 8:(H + 1) * 8], m1B[:, hsl],
                             cs["ones8c"][:, :], start=True, stop=True)
            nc.tensor.matmul(dr0_ps[:, H * 64:(H + 1) * 64], gpsB[:, hsl],
                             cs["wdr0c"][:, :], start=True, stop=True)

        # ---------- Phase B: (mn)-space stream, Pl-major ----------
        tanhSB = sbB.tile([128, 2048], bf, tag="tanhSB")
        tanhGB = sbB.tile([128, 2048], bf, tag="tanhGB")
        q_ps = qgt.tile([128, 512], f32, tag="qps")
        for Pl in range(4):
            psl = slice(Pl * 512, (Pl + 1) * 512)
            S_ps = wps.tile([128, 512], f32, tag="work")
            nc.tensor.matmul(S_ps[:, :], cs[f"bd_w2sym_{Pl}"][:, :], a1B[:, :],
                             start=True, stop=True)
            bs_ps = wps.tile([128, 512], f32, tag="work")
            nc.tensor.matmul(bs_ps[:, :], cs[f"bd_w2sym2_{Pl}"][:, :], cgB[:, :],
                             start=True, stop=True)
            v1_ps = v12.tile([128, 512], f32, tag="v1")
            nc.tensor.matmul(v1_ps[:, :], cs[f"rep1c_{Pl}"][:, :], vTb[:, :],
                             start=True, stop=True)
            v2_ps = v12.tile([128, 512], f32, tag="v2")
            nc.tensor.matmul(v2_ps[:, :], cs[f"rep2c_{Pl}"][:, :], vTb[:, :],
                             start=True, stop=True)

            nc.scalar.activation(tanhSB[:, psl], S_ps[:, :], AF.Tanh,
                                 bias=cs["b2symc"][:, :])
            nc.scalar.activation(tanhGB[:, psl], tanhSB[:, psl], AF.Identity,
                                 bias=cs["g0colB"][:, :])
            sqB = sbP.tile([128, 512], bf, tag="sqB")
            nc.scalar.activation(sqB[:, :], tanhSB[:, psl], AF.Square)
            tanhpB = sbP.tile([128, 512], bf, tag="tanhpB")
            nc.vector.tensor_scalar(out=tanhpB[:, :], in0=sqB[:, :],
                                    scalar1=-1.0, scalar2=1.0,
                                    op0=OP.mult, op1=OP.add)
            vr1b = sbP.tile([128, 512], bf, tag="vr1b")
            nc.vector.tensor_copy(vr1b[:, :], v1_ps[:, :])
            vvTB = sbP.tile([128, 512], bf, tag="vvTB")
            nc.vector.tensor_tensor(out=vvTB[:, :], in0=v2_ps[:, :],
                                    in1=vr1b[:, :], op=OP.mult)
            wtB = sbP.tile([128, 512], bf, tag="wtB")
            nc.vector.tensor_tensor(out=wtB[:, :], in0=bs_ps[:, :],
                                    in1=tanhpB[:, :], op=OP.mult)
            t1preB = sbP.tile([128, 512], bf, tag="t1preB")
            nc.vector.tensor_tensor(out=t1preB[:, :], in0=wtB[:, :],
                                    in1=vr1b[:, :], op=OP.mult)
            ppB = sbP.tile([128, 512], bf, tag="ppB")
            nc.gpsimd.tensor_mul(ppB[:, :], tanhpB[:, :], vvTB[:, :])
            qqB = sbP.tile([128, 512], bf, tag="qqB")
            nc.gpsimd.tensor_mul(qqB[:, :], tanhSB[:, psl], vvTB[:, :])

            nc.tensor.matmul(q_ps[32 * Pl:32 * (Pl + 1), :],
                             cs["bd_w2q"][:, :], ppB[:, :],
                             start=True, stop=True,
                             tile_position=(0, 32 * Pl))
            for H in range(4):
                P = 4 * H + Pl
                hsl = slice(H * 128, (H + 1) * 128)
                nc.tensor.matmul(t1e_ps[:, P * 16:(P + 1) * 16],
                                 t1preB[:, hsl], cs["selc"][:, :],
                                 start=True, stop=True)
                nc.tensor.matmul(qe_ps[:, P * 2:(P + 1) * 2],
                                 qqB[:, hsl], cs["onesc"][:, :],
                                 start=True, stop=True)

        # ---------- q -> gpq -> T2E ----------
        gpqB = sbF.tile([128, 512], bf, tag="gpqB")
        nc.vector.tensor_tensor(out=gpqB[:, :], in0=q_ps[:, :], in1=gpuB[:, :],
                                op=OP.mult)
        for H in range(4):
            nc.tensor.matmul(t2e_ps[:, H * 64:(H + 1) * 64],
                             gpqB[:, H * 128:(H + 1) * 128], cs["w1tc"][:, :],
                             start=True, stop=True)

        # ---------- scalar channel (fp32) ----------
        def stile(tag):
            return sbA.tile([128, 32], f32, tag=tag, name=tag)
        rrawA, sigA, rA, rinvA, kapA, tmpA, uA, absA = (
            stile(t) for t in ["rrawA", "sigA", "rA", "rinvA", "kapA", "tmpA",
                               "uA", "absA"])
        # u = t + br2; softplus(u) = ln(exp(-|u|) + 1) + relu(u)
        nc.scalar.activation(uA[:, :], t_ps[:, :], AF.Identity, bias=br2t[:, :])
        nc.scalar.activation(absA[:, :], t_ps[:, :], AF.Abs, bias=br2t[:, :])
        nc.scalar.activation(absA[:, :], absA[:, :], AF.Exp, scale=-1.0)
        nc.scalar.activation(absA[:, :], absA[:, :], AF.Ln, bias=onet[:, :])
        nc.vector.tensor_scalar_max(rrawA[:, :], uA[:, :], 0.0)
        nc.vector.tensor_add(rrawA[:, :], rrawA[:, :], absA[:, :])
        # sigmoid(u) = 0.5 + 0.5*tanh(u/2)
        nc.scalar.activation(sigA[:, :], t_ps[:, :], AF.Tanh, scale=0.5,
                             bias=br2h[:, :])
        nc.vector.tensor_scalar(out=sigA[:, :], in0=sigA[:, :], scalar1=0.5,
                                scalar2=0.5, op0=OP.mult, op1=OP.add)
        nc.vector.tensor_scalar_max(rA[:, :], rrawA[:, :], 0.1)
        nc.vector.tensor_scalar_min(rA[:, :], rA[:, :], 10.0)
        nc.vector.reciprocal(rinvA[:, :], rA[:, :])
        nc.vector.tensor_scalar(out=kapA[:, :], in0=rrawA[:, :], scalar1=0.1,
                                scalar2=None, op0=OP.is_gt)
        nc.vector.tensor_scalar(out=tmpA[:, :], in0=rrawA[:, :], scalar1=10.0,
                                scalar2=None, op0=OP.is_lt)
        nc.vector.tensor_mul(kapA[:, :], kapA[:, :], tmpA[:, :])
        nc.vector.tensor_mul(kapA[:, :], kapA[:, :], sigA[:, :])

        # ---------- Phase C: gt transposes + LDL k=0 ----------
        gA = sbA.tile([128, 2048], f32, tag="gA", name="gA")
        invdA = sbA.tile([128, 256], f32, tag="invdA")
        wcolA = sbA.tile([128, 224], f32, tag="wcolA")      # (q32, 7)
        tscrA = sbA.tile([128, 1568], f32, tag="tscrA")     # (q32, 49)
        gAv = gA[:, :].rearrange("p (q i j) -> p q i j", i=8, j=8)
        for H in range(4):
            gt_ps = qgt.tile([128, 512], bf, tag="gt")
            for Pl in range(4):
                nc.tensor.matmul(
                    gt_ps[:, Pl * 128:(Pl + 1) * 128],
                    tanhGB[:, Pl * 512 + H * 128:Pl * 512 + (H + 1) * 128],
                    identb[:, :], is_transpose=True, start=True, stop=True)
            q0 = 8 * H                      # first q of this H
            # invd0: diag mn=0
            nc.vector.reciprocal(
                invdA[:, q0:q0 + 8],
                gt_ps[:, :].rearrange("p (q mn) -> p q mn", mn=64)[:, :, 0])
            # wcol0: col0 rows 1..7 -> wcolA[(q in H), 7]
            wv = wcolA[:, 7 * q0:7 * (q0 + 8)].rearrange("p (q i) -> p q i", i=7)
            gtv = gt_ps[:, :].rearrange("p (q i j) -> p q i j", i=8, j=8)
            nc.vector.tensor_copy(wv[:, :, :], gtv[:, :, 1:8, 0])
            # l0 = wcol0 * invd0 -> gA col0
            nc.vector.tensor_tensor(
                out=gAv[:, q0:q0 + 8, 1:8, 0], in0=wv[:, :, :],
                in1=invdA[:, q0:q0 + 8, None].broadcast_to([128, 8, 7]),
                op=OP.mult)
            # outer0 = l0_i * w0_j
            tv = tscrA[:, 49 * q0:49 * (q0 + 8)].rearrange(
                "p (q i j) -> p q i j", i=7, j=7)
            nc.vector.tensor_tensor(
                out=tv[:, :, :, :],
                in0=gAv[:, q0:q0 + 8, 1:8, 0:1].broadcast_to([128, 8, 7, 7]),
                in1=wv[:, :, None, :].broadcast_to([128, 8, 7, 7]),
                op=OP.mult)
            # sub0: gA rect rows1..7 = gt - outer
            nc.vector.tensor_tensor(
                out=gAv[:, q0:q0 + 8, 1:8, 1:8], in0=gtv[:, :, 1:8, 1:8],
                in1=tv[:, :, :, :], op=OP.subtract)

        # ---------- LDL k=1..7 on gA (all 32 q at once) ----------
        wv7 = wcolA[:, :].rearrange("p (q i) -> p q i", i=7)
        tv7 = tscrA[:, :].rearrange("p (q i j) -> p q i j", i=7, j=7)
        for k in range(1, 7):
            m = 7 - k
            nc.vector.reciprocal(invdA[:, 32 * k:32 * (k + 1)], gAv[:, :, k, k])
            nc.vector.tensor_copy(wv7[:, :, :m], gAv[:, :, k + 1:8, k])
            nc.vector.tensor_tensor(
                out=gAv[:, :, k + 1:8, k], in0=wv7[:, :, :m],
                in1=invdA[:, 32 * k:32 * (k + 1), None].broadcast_to([128, 32, m]),
                op=OP.mult)
            nc.vector.tensor_tensor(
                out=tv7[:, :, :m, :m],
                in0=gAv[:, :, k + 1:8, k:k + 1].broadcast_to([128, 32, m, m]),
                in1=wv7[:, :, None, :m].broadcast_to([128, 32, m, m]),
                op=OP.mult)
            nc.vector.tensor_tensor(
                out=gAv[:, :, k + 1:8, k + 1:8], in0=gAv[:, :, k + 1:8, k + 1:8],
                in1=tv7[:, :, :m, :m], op=OP.subtract)
        nc.vector.reciprocal(invdA[:, 224:256], gAv[:, :, 7, 7])

        # ---------- Q, coefZ, z ----------
        qgA, qaA, czA, caA, dvA = (stile(t) for t in
                                   ["qgA", "qaA", "czA", "caA", "dvA"])
        nc.vector.tensor_copy(qgA[:, :], qg_ps[:, :])
        nc.vector.scalar_tensor_tensor(out=qaA[:, :], in0=qe_ps[:, :], scalar=0.1,
                                       in1=qgA[:, :], op0=OP.mult, op1=OP.add)
        nc.vector.tensor_mul(czA[:, :], qaA[:, :], kapA[:, :])
        nc.vector.tensor_mul(czA[:, :], czA[:, :], rinvA[:, :])
        # dv = sum_r dr0*v
        dvmA = sbA.tile([128, 256], f32, tag="dvmA")
        nc.vector.tensor_mul(dvmA[:, :], dr0_ps[:, :], vA[:, :])
        nc.vector.tensor_reduce(
            dvA[:, :], dvmA[:, :].rearrange("p (q r) -> p q r", r=8),
            axis=mybir.AxisListType.X, op=OP.add)
        nc.vector.scalar_tensor_tensor(out=caA[:, :], in0=kapA[:, :], scalar=2.0,
                                       in1=dvA[:, :], op0=OP.mult, op1=OP.mult)
        nc.vector.tensor_mul(caA[:, :], caA[:, :], rinvA[:, :])
        # z = 0.05*T1E - 0.5*T2E - cz*dr0
        t1s = sbA.tile([128, 256], f32, tag="t1s")
        zA = sbA.tile([128, 256], f32, tag="zA")
        nc.vector.tensor_tensor(
            out=t1s[:, :].rearrange("p (q r) -> p q r", r=8),
            in0=dr0_ps.rearrange("p (q r) -> p q r", r=8),
            in1=czA[:, :, None].broadcast_to([128, 32, 8]),
            op=OP.mult)
        nc.vector.scalar_tensor_tensor(out=zA[:, :], in0=t2e_ps[:, :], scalar=-0.5,
                                       in1=t1s[:, :], op0=OP.mult, op1=OP.subtract)
        nc.vector.scalar_tensor_tensor(out=zA[:, :], in0=t1e_ps[:, :], scalar=0.05,
                                       in1=zA[:, :], op0=OP.mult, op1=OP.add)

        # ---------- solve gA y = z ----------
        yv = zA[:, :].rearrange("p (q r) -> p q r", r=8)        # in-place y
        sv = sbA.tile([128, 224], f32, tag="solve_scr")
        svv = sv[:, :].rearrange("p (q i) -> p q i", i=7)
        for k in range(0, 7):
            m = 7 - k
            nc.vector.tensor_tensor(
                out=svv[:, :, :m], in0=gAv[:, :, k + 1:8, k],
                in1=yv[:, :, k:k + 1].broadcast_to([128, 32, m]), op=OP.mult)
            nc.vector.tensor_tensor(
                out=yv[:, :, k + 1:8], in0=yv[:, :, k + 1:8],
                in1=svv[:, :, :m], op=OP.subtract)
        nc.vector.tensor_tensor(
            out=yv[:, :, :],
            in0=yv[:, :, :],
            in1=invdA[:, :].rearrange("p (k q) -> p q k", q=32),
            op=OP.mult)
        for k in range(7, 0, -1):
            nc.vector.tensor_tensor(
                out=svv[:, :, :k], in0=gAv[:, :, k, 0:k],
                in1=yv[:, :, k:k + 1].broadcast_to([128, 32, k]), op=OP.mult)
            nc.vector.tensor_tensor(
                out=yv[:, :, 0:k], in0=yv[:, :, 0:k],
                in1=svv[:, :, :k], op=OP.subtract)

        # ---------- a = -coefA*v - 10*y ----------
        t3 = sbA.tile([128, 256], f32, tag="t3")
        nc.vector.tensor_tensor(
            out=t3[:, :].rearrange("p (q r) -> p q r", r=8),
            in0=vA[:, :].rearrange("p (q r) -> p q r", r=8),
            in1=caA[:, :, None].broadcast_to([128, 32, 8]),
            op=OP.mult)
        nc.vector.scalar_tensor_tensor(out=aA[:, :], in0=zA[:, :], scalar=-10.0,
                                       in1=t3[:, :], op0=OP.mult, op1=OP.subtract)

    # ================= chunk loop =================
    for c in range(NCHUNK):
        xA = sbA.tile([128, 256], f32, tag="xA")
        vA = sbA.tile([128, 256], f32, tag="vA")
        nc.sync.dma_start(out=xA[:, :], in_=dram_chunk(dram["x"], c))
        nc.sync.dma_start(out=vA[:, :], in_=dram_chunk(dram["v"], c))
        xt_ps = transpose32(xA, "xT")
        xT32 = sbF.tile([64, 512], f32, tag="xT32")
        xTb = sbF.tile([64, 512], bf, tag="xTb")
        nc.vector.tensor_copy(xT32[:, :], xt_ps[:64, :])
        nc.vector.tensor_copy(xTb[:, :], xt_ps[:64, :])
        vt_ps = transpose32(vA, "vT")
        vT32 = sbF.tile([64, 512], f32, tag="vT32")
        vTb = sbF.tile([64, 512], bf, tag="vTb")
        nc.vector.tensor_copy(vT32[:, :], vt_ps[:64, :])
        nc.vector.tensor_copy(vTb[:, :], vt_ps[:64, :])

        aA1 = sbA.tile([128, 256], f32, tag="aA1")
        emit_call(xT32, xTb, vTb, vA, aA1)

        vmidA = sbA.tile([128, 256], f32, tag="vmidA")
        nc.vector.scalar_tensor_tensor(out=vmidA[:, :], in0=aA1[:, :], scalar=0.05,
                                       in1=vA[:, :], op0=OP.mult, op1=OP.add)
        xnewA = sbA.tile([128, 256], f32, tag="xnewA")
        nc.vector.scalar_tensor_tensor(out=xnewA[:, :], in0=vmidA[:, :], scalar=0.1,
                                       in1=xA[:, :], op0=OP.mult, op1=OP.add)
        nc.sync.dma_start(out=dram_chunk(dram["x_new"], c), in_=xnewA[:, :])

        xmidT32 = sbF.tile([64, 512], f32, tag="xmidT32")
        nc.vector.scalar_tensor_tensor(out=xmidT32[:, :], in0=vT32[:, :],
                                       scalar=0.05, in1=xT32[:, :],
                                       op0=OP.mult, op1=OP.add)
        xmidTb = sbF.tile([64, 512], bf, tag="xmidTb")
        nc.vector.tensor_copy(xmidTb[:, :], xmidT32[:, :])
        vmidb = sbA.tile([128, 256], bf, tag="vmidb")
        nc.vector.tensor_copy(vmidb[:, :], vmidA[:, :])
        vmidTb = transpose_bf(vmidb, "vmidTb")

        aA2 = sbA.tile([128, 256], f32, tag="aA2")
        emit_call(xmidT32, xmidTb, vmidTb, vmidA, aA2)

        vnewA = sbA.tile([128, 256], f32, tag="vnewA")
        nc.vector.scalar_tensor_tensor(out=vnewA[:, :], in0=aA2[:, :], scalar=0.1,
                                       in1=vA[:, :], op0=OP.mult, op1=OP.add)
        nc.sync.dma_start(out=dram_chunk(dram["v_new"], c), in_=vnewA[:, :])


def _build_module(consts, br2f):
    import concourse.bacc as bacc
    import concourse.mybir as mybir
    import concourse.tile as tile
    from contextlib import ExitStack

    f32 = mybir.dt.float32
    bf = mybir.dt.bfloat16
    nc = bacc.Bacc("TRN2", target_bir_lowering=False, debug=False,
                   num_devices=NCORES)
    dram = {}
    dram["x"] = nc.dram_tensor("x", [NTOK, D], f32, kind="ExternalInput").ap()
    dram["v"] = nc.dram_tensor("v", [NTOK, D], f32, kind="ExternalInput").ap()
    for name, arr in consts.items():
        dt = bf if arr.dtype == bfloat16 else f32
        dram[name] = nc.dram_tensor(name, list(arr.shape), dt,
                                    kind="ExternalInput").ap()
    dram["x_new"] = nc.dram_tensor("x_new", [NTOK, D], f32,
                                   kind="ExternalOutput").ap()
    dram["v_new"] = nc.dram_tensor("v_new", [NTOK, D], f32,
                                   kind="ExternalOutput").ap()
    with tile.TileContext(nc) as tc:
        with ExitStack() as ctx:
            _emit(nc, tc, ctx, dram, br2f)
    nc.compile()
    return nc


def kernel(x, v, L, W1, b1, W2, b2, Wr1, br1, Wr2, br2):
    x = np.ascontiguousarray(np.asarray(x, dtype=np.float32))
    v = np.ascontiguousarray(np.asarray(v, dtype=np.float32))
    consts, br2f = _build_consts(L, W1, b1, W2, b2, Wr1, br1, Wr2, br2)
    nc = _build_module(consts, br2f)

    from concourse.bass_utils import run_bass_kernel_spmd
    in_maps = []
    for c in range(NCORES):
        m = {"x": np.ascontiguousarray(x[c]), "v": np.ascontiguousarray(v[c])}
        m.update(consts)
        in_maps.append(m)
    import os as _os
    trace = _os.environ.get("KERNEL_TRACE", "0") == "1"
    tmpdir = _os.environ.get("KERNEL_TRACE_DIR") or None
    res = run_bass_kernel_spmd(nc, in_maps, core_ids=list(range(NCORES)),
                               trace=trace, tmpdir=tmpdir)
    global LAST_EXEC_TIME_NS, LAST_TRACE
    LAST_EXEC_TIME_NS = res.exec_time_ns
    LAST_TRACE = res.instructions_and_trace
    x_new = np.stack([r["x_new"] for r in res.results]).astype(np.float32)
    v_new = np.stack([r["v_new"] for r in res.results]).astype(np.float32)
    return (x_new, v_new)


# revision 11
# speedup vs baseline: 1.9294x; 1.0069x over previous
"""Trainium2 Bass kernel for nn_CognitiveManifold (geodesic RK2 step).

8 NeuronCores, pure data parallel: 8192 tokens/core, full inputs in, full
outputs out. Analytic metric derivatives + one 8x8 SPD LDL^T solve per
token. v3: bf16 tensor-engine path (fp32 kept for the clip-sensitive
R-channel and the LDL/solve), single 8192-token chunk so every A-layout
vector op covers 64 q (half the instruction count of v2), LDL k0 batched
from an SBUF-staged metric, copies/casts on the Scalar engine.

Layouts (token = 64*p + q, q = 32h + 8H' + q3 = 32h + 8H' + 2Pl + qs):
  A (tokens on partitions): [128, (q=64, feat)] fp32
  B (features on partitions, tokens on free):
    (d)-space  [64  = 8*q3+d,   (H=8, 128p)]
    (j)-space  [128 = 16*q3+j,  (H=8, 128p)]
    (mn)-space [128 = 64*qs+mn, (h2|Pl4|H'4|128p)]
"""

import numpy as np
from ml_dtypes import bfloat16

try:  # concourse ships with the container; ensure it's importable
    import concourse  # noqa: F401
except ImportError:  # pragma: no cover
    import sys as _sys
    for _p in ("/opt/trn_rl_repo", "/root/.axon_site/_ro/trn_rl_repo"):
        if _p not in _sys.path:
            _sys.path.insert(0, _p)

LAST_EXEC_TIME_NS = None
LAST_TRACE = None
D = 8
NCORES = 8
NTOK = 8192
NP = 128
NQ = NTOK // NP    # 64


def _build_consts(L, W1, b1, W2, b2, Wr1, br1, Wr2, br2):
    L, W1, b1, W2, b2 = (np.asarray(a, np.float64) for a in (L, W1, b1, W2, b2))
    Wr1, br1, Wr2, br2 = (np.asarray(a, np.float64) for a in (Wr1, br1, Wr2, br2))
    G0 = L @ L.T + 1e-4 * np.eye(D)
    W2r = W2.reshape(16, D, D)
    W2sym = (0.5 * (W2r + np.swapaxes(W2r, 1, 2))).reshape(16, 64)
    b2r = b2.reshape(D, D)
    b2sym = (0.5 * (b2r + b2r.T)).reshape(64)
    W2sym2 = (W2r + np.swapaxes(W2r, 1, 2)).reshape(16, 64)
    Wdr0 = Wr1 * Wr2[:, 0][None, :]          # [r, j] = Wr1[r,j]*Wr2[j,0]

    def blockdiag(w, g):
        kin, mout = w.shape
        out = np.zeros((g * kin, g * mout), dtype=np.float64)
        for i in range(g):
            out[i * kin:(i + 1) * kin, i * mout:(i + 1) * mout] = w
        return out

    B = {}   # bf16 consts
    F = {}   # fp32 consts
    B["eye128b"] = np.eye(128)
    F["eye128"] = np.eye(128)
    B["bd_w1"] = blockdiag(W1, 8)                 # [64,128]
    F["bd_wr1"] = blockdiag(Wr1, 8)               # [64,64] fp32 (R-channel)
    B["bd_g0"] = blockdiag(G0, 8)                 # [64,64]
    for Pl in range(4):
        w = np.zeros((128, 128))
        w2 = np.zeros((128, 128))
        for qs in range(2):
            q3 = 2 * Pl + qs
            w[q3 * 16:(q3 + 1) * 16, qs * 64:(qs + 1) * 64] = W2sym
            w2[q3 * 16:(q3 + 1) * 16, qs * 64:(qs + 1) * 64] = W2sym2
        B[f"bd_w2sym_{Pl}"] = w
        B[f"bd_w2sym2_{Pl}"] = w2
    B["bd_w2q"] = blockdiag(0.1 * W2.T, 2)        # [128,32]
    sel = np.zeros((128, 16))
    for qs in range(2):
        for n in range(D):
            for r in range(D):
                sel[qs * 64 + n * D + r, qs * D + r] = 1.0
    B["selc"] = sel
    ones2 = np.zeros((128, 2))
    ones2[:64, 0] = 1.0
    ones2[64:, 1] = 1.0
    B["onesc"] = ones2
    ones8 = np.zeros((64, 8))
    for q3 in range(8):
        ones8[q3 * D:(q3 + 1) * D, q3] = 1.0
    B["ones8c"] = ones8
    B["w1tc"] = blockdiag(W1.T, 8)                # [128,64]
    B["wdr0c"] = blockdiag(Wdr0.T, 8)             # [64,64]
    wr2c = np.zeros((64, 8))
    for q3 in range(8):
        wr2c[q3 * D:(q3 + 1) * D, q3] = Wr2[:, 0]
    F["wr2c"] = wr2c                              # fp32 (R-channel)
    for Pl in range(4):
        rep1 = np.zeros((64, 128))
        rep2 = np.zeros((64, 128))
        for qs in range(2):
            q3 = 2 * Pl + qs
            for d in range(D):
                for r in range(D):
                    rep1[q3 * D + d, qs * 64 + d * D + r] = 1.0   # n = d
                    rep2[q3 * D + d, qs * 64 + r * D + d] = 1.0   # r = d
        B[f"rep1c_{Pl}"] = rep1
        B[f"rep2c_{Pl}"] = rep2
    F["b1c"] = np.tile(b1, 8)                     # [128]
    F["br1c"] = np.tile(br1, 8)                   # [64]
    F["b2symc"] = np.tile(b2sym, 2)               # [128]
    F["g0colB"] = np.tile((10.0 * G0).reshape(64), 2)  # [128]
    consts = {k: np.ascontiguousarray(v, dtype=bfloat16) for k, v in B.items()}
    consts.update({k: np.ascontiguousarray(v, dtype=np.float32)
                   for k, v in F.items()})
    return consts, float(br2[0])


CONST_DTYPES = dict(
    **{k: "bf16" for k in
       ["eye128b", "bd_w1", "bd_g0", "bd_w2q", "selc", "onesc", "ones8c",
        "w1tc", "wdr0c"]
       + [f"bd_w2sym_{p}" for p in range(4)]
       + [f"bd_w2sym2_{p}" for p in range(4)]
       + [f"rep1c_{p}" for p in range(4)]
       + [f"rep2c_{p}" for p in range(4)]},
    **{k: "f32" for k in
       ["eye128", "bd_wr1", "wr2c", "b1c", "br1c", "b2symc", "g0colB"]},
)

CONST_SHAPES = {
    "eye128b": (128, 128), "eye128": (128, 128),
    "bd_w1": (64, 128), "bd_wr1": (64, 64), "bd_g0": (64, 64),
    "bd_w2q": (128, 32),
    "selc": (128, 16), "onesc": (128, 2), "ones8c": (64, 8),
    "w1tc": (128, 64), "wdr0c": (64, 64), "wr2c": (64, 8),
    "b1c": (128,), "br1c": (64,), "b2symc": (128,), "g0colB": (128,),
    **{f"bd_w2sym_{p}": (128, 128) for p in range(4)},
    **{f"bd_w2sym2_{p}": (128, 128) for p in range(4)},
    **{f"rep1c_{p}": (64, 128) for p in range(4)},
    **{f"rep2c_{p}": (64, 128) for p in range(4)},
}


def _emit(nc, tc, ctx, dram, br2f):
    import concourse.mybir as mybir

    f32 = mybir.dt.float32
    bf = mybir.dt.bfloat16
    AF = mybir.ActivationFunctionType
    OP = mybir.AluOpType

    consts = ctx.enter_context(tc.tile_pool(name="consts", bufs=1))
    sbB = ctx.enter_context(tc.tile_pool(name="sbB", bufs=1))    # big bf16 B
    sbP = ctx.enter_context(tc.tile_pool(name="sbP", bufs=2))    # per-Pl bf16
    sbF = ctx.enter_context(tc.tile_pool(name="sbF", bufs=1))    # fwd B tiles
    sbA = ctx.enter_context(tc.tile_pool(name="sbA", bufs=1))    # A-layout f32
    wps = ctx.enter_context(tc.tile_pool(name="wps", bufs=1, space="PSUM"))
    v12 = ctx.enter_context(tc.tile_pool(name="v12", bufs=1, space="PSUM"))
    qgt = ctx.enter_context(tc.tile_pool(name="qgt", bufs=1, space="PSUM"))
    scps = ctx.enter_context(tc.tile_pool(name="scps", bufs=1, space="PSUM"))

    cs = {}
    for name, shape in CONST_SHAPES.items():
        dt = bf if CONST_DTYPES[name] == "bf16" else f32
        if len(shape) == 1:
            t = consts.tile([shape[0], 1], dt, name=name, tag=name)
            nc.sync.dma_start(out=t[:, :],
                              in_=dram[name].rearrange("(p one) -> p one", one=1))
        else:
            t = consts.tile(list(shape), dt, name=name, tag=name)
            nc.sync.dma_start(out=t[:, :], in_=dram[name][:, :])
        cs[name] = t
    identb = cs["eye128b"]
    ident32 = cs["eye128"]
    br2t = consts.tile([128, 1], f32, name="br2t")
    nc.vector.memset(br2t[:, :], br2f)
    br2h = consts.tile([128, 1], f32, name="br2h")
    nc.vector.memset(br2h[:, :], 0.5 * br2f)
    onet = consts.tile([128, 1], f32, name="onet")
    nc.vector.memset(onet[:, :], 1.0)

    def dram_full(t):
        return t[:, :].rearrange("(p q) d -> p (q d)", q=NQ)

    def transpose32(src, f32tag, bftag):
        """[128, 512] A-(q64,d8) fp32 -> xT32 [64,1024] f32 + xTb bf16 SBUF."""
        o32 = sbF.tile([64, 1024], f32, tag=f32tag)
        ob = sbF.tile([64, 1024], bf, tag=bftag)
        for half in range(2):
            pt = wps.tile([128, 512], f32, tag="S")
            for Hp in range(4):
                H = 4 * half + Hp
                nc.tensor.matmul(pt[:64, Hp * 128:(Hp + 1) * 128],
                                 src[:, H * 64:(H + 1) * 64],
                                 ident32[:, :], is_transpose=True,
                                 start=True, stop=True)
            hsl = slice(half * 512, (half + 1) * 512)
            nc.scalar.activation(o32[:, hsl], pt[:64, :], AF.Identity)
            nc.scalar.activation(ob[:, hsl], pt[:64, :], AF.Identity)
        return o32, ob

    def transpose_bf(src, tag):
        """[128, 512] A-(q64,d8) bf16 -> [64, 1024] bf16 SBUF."""
        out = sbF.tile([64, 1024], bf, tag=tag)
        for half in range(2):
            pt = qgt.tile([128, 512], bf, tag="gt")
            for Hp in range(4):
                H = 4 * half + Hp
                nc.tensor.matmul(pt[:64, Hp * 128:(Hp + 1) * 128],
                                 src[:, H * 64:(H + 1) * 64],
                                 identb[:, :], is_transpose=True,
                                 start=True, stop=True)
            nc.scalar.activation(out[:, half * 512:(half + 1) * 512],
                                 pt[:64, :], AF.Identity)
        return out

    def emit_call(xT32, xTb, vTb, vA, aA):
        """One christoffel+contraction; writes acceleration into aA [128,(q64,8)]."""

        # ---------- Phase A: forward matmuls + activations (per half) ----------
        a1B = sbF.tile([128, 1024], bf, tag="a1B")
        gpuB = sbF.tile([128, 1024], bf, tag="gpuB")
        a2B = sbF.tile([64, 1024], f32, tag="a2B")
        gpsB = sbF.tile([64, 1024], bf, tag="gpsB")
        cgB = sbF.tile([128, 1024], bf, tag="cgB")
        m1B = sbF.tile([64, 1024], bf, tag="m1B")
        for h in range(2):
            hs = slice(h * 512, (h + 1) * 512)
            u_ps = wps.tile([128, 512], f32, tag="S")
            nc.tensor.matmul(u_ps[:, :], cs["bd_w1"][:, :], xTb[:, hs],
                             start=True, stop=True)
            nc.scalar.activation(a1B[:, hs], u_ps[:, :], AF.Gelu,
                                 bias=cs["b1c"][:, :])
            nc.scalar.activation(gpuB[:, hs], u_ps[:, :], AF.Derivative_Gelu,
                                 bias=cs["b1c"][:, :])
            s_ps = wps.tile([128, 512], f32, tag="bs")
            nc.tensor.matmul(s_ps[:64, :], cs["bd_wr1"][:, :], xT32[:, hs],
                             start=True, stop=True)
            nc.scalar.activation(a2B[:, hs], s_ps[:64, :], AF.Gelu,
                                 bias=cs["br1c"][:, :])
            nc.scalar.activation(gpsB[:, hs], s_ps[:64, :], AF.Derivative_Gelu,
                                 bias=cs["br1c"][:, :])
            c_ps = wps.tile([128, 512], f32, tag="S")
            nc.tensor.matmul(c_ps[:, :], cs["bd_w1"][:, :], vTb[:, hs],
                             start=True, stop=True)
            nc.vector.tensor_tensor(out=cgB[:, hs], in0=c_ps[:, :],
                                    in1=gpuB[:, hs], op=OP.mult)
            gv_ps = wps.tile([128, 512], f32, tag="bs")
            nc.tensor.matmul(gv_ps[:64, :], cs["bd_g0"][:, :], vTb[:, hs],
                             start=True, stop=True)
            nc.vector.tensor_tensor(out=m1B[:, hs], in0=gv_ps[:64, :],
                                    in1=vTb[:, hs], op=OP.mult)

        # ---------- Phase B: (mn)-stream + packs, per (half, Pl) ----------
        # pack_h psum [128,1024]: [0:32)t | [32:64)QG | [64:96)QE |
        #   [128:384)dr0 | [384:640)T1E | [640:896)T2E   (local P' = 4H'+Pl)
        pkSB = sbA.tile([128, 2048], f32, tag="pkSB", name="pkSB")
        tanhSB = sbB.tile([128, 4096], bf, tag="tanhSB")
        tanhGB = sbB.tile([128, 4096], bf, tag="tanhGB")
        gpqB = sbF.tile([128, 1024], bf, tag="gpqB")
        for h in range(2):
            hs = slice(h * 512, (h + 1) * 512)
            pk = scps.tile([128, 1024], f32, tag="pack")
            for Hp in range(4):
                hsl = slice(h * 512 + Hp * 128, h * 512 + (Hp + 1) * 128)
                nc.tensor.matmul(pk[:, Hp * 8:(Hp + 1) * 8], a2B[:, hsl],
                                 cs["wr2c"][:, :], start=True, stop=True)
                nc.tensor.matmul(pk[:, 32 + Hp * 8:32 + (Hp + 1) * 8],
                                 m1B[:, hsl], cs["ones8c"][:, :],
                                 start=True, stop=True)
                nc.tensor.matmul(pk[:, 128 + Hp * 64:128 + (Hp + 1) * 64],
                                 gpsB[:, hsl], cs["wdr0c"][:, :],
                                 start=True, stop=True)
            q_ps = qgt.tile([128, 512], f32, tag="qps")
            for Pl in range(4):
                psl = slice(h * 2048 + Pl * 512, h * 2048 + (Pl + 1) * 512)
                S_ps = wps.tile([128, 512], f32, tag="S")
                nc.tensor.matmul(S_ps[:, :], cs[f"bd_w2sym_{Pl}"][:, :],
                                 a1B[:, hs], start=True, stop=True)
                bs_ps = wps.tile([128, 512], f32, tag="bs")
                nc.tensor.matmul(bs_ps[:, :], cs[f"bd_w2sym2_{Pl}"][:, :],
                                 cgB[:, hs], start=True, stop=True)
                v1_ps = v12.tile([128, 512], f32, tag="v1")
                nc.tensor.matmul(v1_ps[:, :], cs[f"rep1c_{Pl}"][:, :],
                                 vTb[:, hs], start=True, stop=True)
                v2_ps = v12.tile([128, 512], f32, tag="v2")
                nc.tensor.matmul(v2_ps[:, :], cs[f"rep2c_{Pl}"][:, :],
                                 vTb[:, hs], start=True, stop=True)

                nc.scalar.activation(tanhSB[:, psl], S_ps[:, :], AF.Tanh,
                                     bias=cs["b2symc"][:, :])
                nc.scalar.activation(tanhGB[:, psl], tanhSB[:, psl],
                                     AF.Identity, bias=cs["g0colB"][:, :])
                sqB = sbP.tile([128, 512], bf, tag="sqB")
                nc.scalar.activation(sqB[:, :], tanhSB[:, psl], AF.Square)
                tanhpB = sbP.tile([128, 512], bf, tag="tanhpB")
                nc.vector.tensor_scalar(out=tanhpB[:, :], in0=sqB[:, :],
                                        scalar1=-1.0, scalar2=1.0,
                                        op0=OP.mult, op1=OP.add)
                vr1b = sbP.tile([128, 512], bf, tag="vr1b")
                nc.scalar.activation(vr1b[:, :], v1_ps[:, :], AF.Identity)
                vvTB = sbP.tile([128, 512], bf, tag="vvTB")
                nc.vector.tensor_tensor(out=vvTB[:, :], in0=v2_ps[:, :],
                                        in1=vr1b[:, :], op=OP.mult)
                wtB = sbP.tile([128, 512], bf, tag="wtB")
                nc.vector.tensor_tensor(out=wtB[:, :], in0=bs_ps[:, :],
                                        in1=tanhpB[:, :], op=OP.mult)
                t1preB = sbP.tile([128, 512], bf, tag="t1preB")
                nc.vector.tensor_tensor(out=t1preB[:, :], in0=wtB[:, :],
                                        in1=vr1b[:, :], op=OP.mult)
                ppB = sbP.tile([128, 512], bf, tag="ppB")
                nc.gpsimd.tensor_mul(ppB[:, :], tanhpB[:, :], vvTB[:, :])
                qqB = sbP.tile([128, 512], bf, tag="qqB")
                nc.gpsimd.tensor_mul(qqB[:, :], tanhSB[:, psl], vvTB[:, :])

                nc.tensor.matmul(q_ps[32 * Pl:32 * (Pl + 1), :],
                                 cs["bd_w2q"][:, :], ppB[:, :],
                                 start=True, stop=True,
                                 tile_position=(0, 32 * Pl))
                for Hp in range(4):
                    Px = 4 * Hp + Pl
                    hpl = slice(Hp * 128, (Hp + 1) * 128)
                    nc.tensor.matmul(pk[:, 384 + Px * 16:384 + (Px + 1) * 16],
                                     t1preB[:, hpl], cs["selc"][:, :],
                                     start=True, stop=True)
                    nc.tensor.matmul(pk[:, 64 + Px * 2:64 + (Px + 1) * 2],
                                     qqB[:, hpl], cs["onesc"][:, :],
                                     start=True, stop=True)
            # q -> gpq -> T2E (this half)
            nc.vector.tensor_tensor(out=gpqB[:, hs], in0=q_ps[:, :],
                                    in1=gpuB[:, hs], op=OP.mult)
            for Hp in range(4):
                nc.tensor.matmul(pk[:, 640 + Hp * 64:640 + (Hp + 1) * 64],
                                 gpqB[:, h * 512 + Hp * 128:
                                      h * 512 + (Hp + 1) * 128],
                                 cs["w1tc"][:, :], start=True, stop=True)
            # stage the pack to SBUF, freeing the psum bank pair
            nc.scalar.activation(pkSB[:, h * 1024:(h + 1) * 1024], pk[:, :],
                                 AF.Identity)

        # A-layout views of the staged pack (q = 32h + q_local)
        pk2 = pkSB[:, :].rearrange("p (h c) -> p h c", h=2)
        t_v = pk2[:, :, 0:32]
        qg_v = pk2[:, :, 32:64]
        qe_v = pk2[:, :, 64:96]
        dr0_v = pk2[:, :, 128:384]
        t1e_v = pk2[:, :, 384:640]
        t2e_v = pk2[:, :, 640:896]

        # ---------- scalar channel (fp32) ----------
        def stile(tag):
            return sbA.tile([128, 64], f32, tag=tag, name=tag)
        rrawA, sigA, rA, rinvA, kapA, tmpA, uA, absA = (
            stile(t) for t in ["rrawA", "sigA", "rA", "rinvA", "kapA", "tmpA",
                               "uA", "absA"])
        uA2 = uA[:, :].rearrange("p (h c) -> p h c", h=2)
        absA2 = absA[:, :].rearrange("p (h c) -> p h c", h=2)
        sigA2 = sigA[:, :].rearrange("p (h c) -> p h c", h=2)
        # u = t + br2; softplus(u) = ln(exp(-|u|) + 1) + relu(u)
        nc.scalar.activation(uA2[:, :, :], t_v, AF.Identity, bias=br2t[:, :])
        nc.scalar.activation(absA2[:, :, :], t_v, AF.Abs, bias=br2t[:, :])
        nc.scalar.activation(absA[:, :], absA[:, :], AF.Exp, scale=-1.0)
        nc.scalar.activation(absA[:, :], absA[:, :], AF.Ln, bias=onet[:, :])
        nc.vector.tensor_scalar_max(rrawA[:, :], uA[:, :], 0.0)
        nc.vector.tensor_add(rrawA[:, :], rrawA[:, :], absA[:, :])
        # sigmoid(u) = 0.5 + 0.5*tanh(u/2)
        nc.scalar.activation(sigA2[:, :, :], t_v, AF.Tanh, scale=0.5,
                             bias=br2h[:, :])
        nc.vector.tensor_scalar(out=sigA[:, :], in0=sigA[:, :], scalar1=0.5,
                                scalar2=0.5, op0=OP.mult, op1=OP.add)
        nc.vector.tensor_scalar_max(rA[:, :], rrawA[:, :], 0.1)
        nc.vector.tensor_scalar_min(rA[:, :], rA[:, :], 10.0)
        nc.vector.reciprocal(rinvA[:, :], rA[:, :])
        nc.vector.tensor_scalar(out=kapA[:, :], in0=rrawA[:, :], scalar1=0.1,
                                scalar2=None, op0=OP.is_gt)
        nc.vector.tensor_scalar(out=tmpA[:, :], in0=rrawA[:, :], scalar1=10.0,
                                scalar2=None, op0=OP.is_lt)
        nc.vector.tensor_mul(kapA[:, :], kapA[:, :], tmpA[:, :])
        nc.vector.tensor_mul(kapA[:, :], kapA[:, :], sigA[:, :])

        # ---------- Phase C: gt transposes -> gtSB, batched LDL k=0 ----------
        gtSB = sbB.tile([128, 4096], bf, tag="gtSB")
        for h in range(2):
            for Hp in range(4):
                H = 4 * h + Hp
                gt_ps = qgt.tile([128, 512], bf, tag="gt")
                for Pl in range(4):
                    nc.tensor.matmul(
                        gt_ps[:, Pl * 128:(Pl + 1) * 128],
                        tanhGB[:, h * 2048 + Pl * 512 + Hp * 128:
                               h * 2048 + Pl * 512 + (Hp + 1) * 128],
                        identb[:, :], is_transpose=True, start=True, stop=True)
                nc.scalar.activation(gtSB[:, H * 512:(H + 1) * 512],
                                     gt_ps[:, :], AF.Identity)

        gA = sbA.tile([128, 4096], f32, tag="gA", name="gA")
        invdA = sbA.tile([128, 512], f32, tag="invdA")
        wcolA = sbA.tile([128, 448], f32, tag="wcolA")      # (q64, 7)
        tscrA = sbA.tile([128, 3136], f32, tag="tscrA")     # (q64, 49)
        gAv = gA[:, :].rearrange("p (q i j) -> p q i j", i=8, j=8)
        gtv = gtSB[:, :].rearrange("p (q i j) -> p q i j", i=8, j=8)
        wv = wcolA[:, :].rearrange("p (q i) -> p q i", i=7)
        tv = tscrA[:, :].rearrange("p (q i j) -> p q i j", i=7, j=7)
        nc.vector.reciprocal(invdA[:, 0:64], gtv[:, :, 0, 0])
        nc.vector.tensor_copy(wv[:, :, :], gtv[:, :, 1:8, 0])
        nc.vector.tensor_tensor(
            out=gAv[:, :, 1:8, 0], in0=wv[:, :, :],
            in1=invdA[:, 0:64, None].broadcast_to([128, 64, 7]), op=OP.mult)
        nc.vector.tensor_tensor(
            out=tv[:, :, :, :],
            in0=gAv[:, :, 1:8, 0:1].broadcast_to([128, 64, 7, 7]),
            in1=wv[:, :, None, :].broadcast_to([128, 64, 7, 7]), op=OP.mult)
        nc.vector.tensor_tensor(
            out=gAv[:, :, 1:8, 1:8], in0=gtv[:, :, 1:8, 1:8],
            in1=tv[:, :, :, :], op=OP.subtract)

        # ---------- LDL k=1..7 on gA (all 64 q at once) ----------
        for k in range(1, 7):
            m = 7 - k
            nc.vector.reciprocal(invdA[:, 64 * k:64 * (k + 1)], gAv[:, :, k, k])
            nc.vector.tensor_copy(wv[:, :, :m], gAv[:, :, k + 1:8, k])
            nc.vector.tensor_tensor(
                out=gAv[:, :, k + 1:8, k], in0=wv[:, :, :m],
                in1=invdA[:, 64 * k:64 * (k + 1), None].broadcast_to([128, 64, m]),
                op=OP.mult)
            nc.vector.tensor_tensor(
                out=tv[:, :, :m, :m],
                in0=gAv[:, :, k + 1:8, k:k + 1].broadcast_to([128, 64, m, m]),
                in1=wv[:, :, None, :m].broadcast_to([128, 64, m, m]),
                op=OP.mult)
            nc.vector.tensor_tensor(
                out=gAv[:, :, k + 1:8, k + 1:8], in0=gAv[:, :, k + 1:8, k + 1:8],
                in1=tv[:, :, :m, :m], op=OP.subtract)
        nc.vector.reciprocal(invdA[:, 448:512], gAv[:, :, 7, 7])

        # ---------- Q, coefZ, z ----------
        qaA, czA, caA, dvA = (stile(t) for t in ["qaA", "czA", "caA", "dvA"])
        qaA2 = qaA[:, :].rearrange("p (h c) -> p h c", h=2)
        nc.vector.scalar_tensor_tensor(out=qaA2[:, :, :], in0=qe_v, scalar=0.1,
                                       in1=qg_v, op0=OP.mult, op1=OP.add)
        nc.vector.tensor_mul(czA[:, :], qaA[:, :], kapA[:, :])
        nc.vector.tensor_mul(czA[:, :], czA[:, :], rinvA[:, :])
        # dv = sum_r dr0*v
        dvmA = sbA.tile([128, 512], f32, tag="dvmA")
        dvmA2 = dvmA[:, :].rearrange("p (h c) -> p h c", h=2)
        nc.vector.tensor_tensor(out=dvmA2[:, :, :], in0=dr0_v,
                                in1=vA[:, :].rearrange("p (h c) -> p h c", h=2),
                                op=OP.mult)
        nc.vector.tensor_reduce(
            dvA[:, :], dvmA[:, :].rearrange("p (q r) -> p q r", r=8),
            axis=mybir.AxisListType.X, op=OP.add)
        nc.vector.scalar_tensor_tensor(out=caA[:, :], in0=kapA[:, :], scalar=2.0,
                                       in1=dvA[:, :], op0=OP.mult, op1=OP.mult)
        nc.vector.tensor_mul(caA[:, :], caA[:, :], rinvA[:, :])
        # z = 0.05*T1E - 0.5*T2E - cz*dr0
        t1s = sbA.tile([128, 512], f32, tag="t1s")
        zA = sbA.tile([128, 512], f32, tag="zA")
        nc.vector.tensor_tensor(
            out=t1s[:, :].rearrange("p (h q r) -> p h q r", h=2, r=8),
            in0=dr0_v.rearrange("p h (q r) -> p h q r", r=8),
            in1=czA[:, :].rearrange("p (h q) -> p h q", h=2)[:, :, :, None]
                .broadcast_to([128, 2, 32, 8]),
            op=OP.mult)
        zA2 = zA[:, :].rearrange("p (h c) -> p h c", h=2)
        nc.vector.scalar_tensor_tensor(
            out=zA2[:, :, :], in0=t2e_v, scalar=-0.5,
            in1=t1s[:, :].rearrange("p (h c) -> p h c", h=2),
            op0=OP.mult, op1=OP.subtract)
        nc.vector.scalar_tensor_tensor(
            out=zA2[:, :, :], in0=t1e_v, scalar=0.05,
            in1=zA2[:, :, :], op0=OP.mult, op1=OP.add)

        # ---------- solve gA y = z ----------
        yv = zA[:, :].rearrange("p (q r) -> p q r", r=8)        # in-place y
        sv = sbA.tile([128, 448], f32, tag="solve_scr")
        svv = sv[:, :].rearrange("p (q i) -> p q i", i=7)
        for k in range(0, 7):
            m = 7 - k
            nc.vector.tensor_tensor(
                out=svv[:, :, :m], in0=gAv[:, :, k + 1:8, k],
                in1=yv[:, :, k:k + 1].broadcast_to([128, 64, m]), op=OP.mult)
            nc.vector.tensor_tensor(
                out=yv[:, :, k + 1:8], in0=yv[:, :, k + 1:8],
                in1=svv[:, :, :m], op=OP.subtract)
        nc.vector.tensor_tensor(
            out=yv[:, :, :],
            in0=yv[:, :, :],
            in1=invdA[:, :].rearrange("p (k q) -> p q k", q=64),
            op=OP.mult)
        for k in range(7, 0, -1):
            nc.vector.tensor_tensor(
                out=svv[:, :, :k], in0=gAv[:, :, k, 0:k],
                in1=yv[:, :, k:k + 1].broadcast_to([128, 64, k]), op=OP.mult)
            nc.vector.tensor_tensor(
                out=yv[:, :, 0:k], in0=yv[:, :, 0:k],
                in1=svv[:, :, :k], op=OP.subtract)

        # ---------- a = -coefA*v - 10*y ----------
        t3 = sbA.tile([128, 512], f32, tag="t3")
        nc.vector.tensor_tensor(
            out=t3[:, :].rearrange("p (q r) -> p q r", r=8),
            in0=vA[:, :].rearrange("p (q r) -> p q r", r=8),
            in1=caA[:, :, None].broadcast_to([128, 64, 8]),
            op=OP.mult)
        nc.vector.scalar_tensor_tensor(out=aA[:, :], in0=zA[:, :], scalar=-10.0,
                                       in1=t3[:, :], op0=OP.mult, op1=OP.subtract)

    # ================= main =================
    xA = sbA.tile([128, 512], f32, tag="xA")
    vA = sbA.tile([128, 512], f32, tag="vA")
    nc.sync.dma_start(out=xA[:, :], in_=dram_full(dram["x"]))
    nc.sync.dma_start(out=vA[:, :], in_=dram_full(dram["v"]))
    xT32, xTb = transpose32(xA, "xT32", "xTb")
    vT32, vTb = transpose32(vA, "vT32", "vTb")

    aA1 = sbA.tile([128, 512], f32, tag="aA1")
    emit_call(xT32, xTb, vTb, vA, aA1)

    vmidA = sbA.tile([128, 512], f32, tag="vmidA")
    nc.vector.scalar_tensor_tensor(out=vmidA[:, :], in0=aA1[:, :], scalar=0.05,
                                   in1=vA[:, :], op0=OP.mult, op1=OP.add)
    xnewA = sbA.tile([128, 512], f32, tag="xnewA")
    nc.vector.scalar_tensor_tensor(out=xnewA[:, :], in0=vmidA[:, :], scalar=0.1,
                                   in1=xA[:, :], op0=OP.mult, op1=OP.add)
    nc.sync.dma_start(out=dram_full(dram["x_new"]), in_=xnewA[:, :])

    xmidT32 = sbF.tile([64, 1024], f32, tag="xmidT32")
    nc.vector.scalar_tensor_tensor(out=xmidT32[:, :], in0=vT32[:, :],
                                   scalar=0.05, in1=xT32[:, :],
                                   op0=OP.mult, op1=OP.add)
    xmidTb = sbF.tile([64, 1024], bf, tag="xmidTb")
    nc.scalar.activation(xmidTb[:, :], xmidT32[:, :], AF.Identity)
    vmidb = sbA.tile([128, 512], bf, tag="vmidb")
    nc.scalar.activation(vmidb[:, :], vmidA[:, :], AF.Identity)
    vmidTb = transpose_bf(vmidb, "vmidTb")

    aA2 = sbA.tile([128, 512], f32, tag="aA2")
    emit_call(xmidT32, xmidTb, vmidTb, vmidA, aA2)

    vnewA = sbA.tile([128, 512], f32, tag="vnewA")
    nc.vector.scalar_tensor_tensor(out=vnewA[:, :], in0=aA2[:, :], scalar=0.1,
                                   in1=vA[:, :], op0=OP.mult, op1=OP.add)
    nc.sync.dma_start(out=dram_full(dram["v_new"]), in_=vnewA[:, :])


def _build_module(consts, br2f):
    import concourse.bacc as bacc
    import concourse.mybir as mybir
    import concourse.tile as tile
    from contextlib import ExitStack

    f32 = mybir.dt.float32
    bf = mybir.dt.bfloat16
    nc = bacc.Bacc("TRN2", target_bir_lowering=False, debug=False,
                   num_devices=NCORES)
    dram = {}
    dram["x"] = nc.dram_tensor("x", [NTOK, D], f32, kind="ExternalInput").ap()
    dram["v"] = nc.dram_tensor("v", [NTOK, D], f32, kind="ExternalInput").ap()
    for name, arr in consts.items():
        dt = bf if arr.dtype == bfloat16 else f32
        dram[name] = nc.dram_tensor(name, list(arr.shape), dt,
                                    kind="ExternalInput").ap()
    dram["x_new"] = nc.dram_tensor("x_new", [NTOK, D], f32,
                                   kind="ExternalOutput").ap()
    dram["v_new"] = nc.dram_tensor("v_new", [NTOK, D], f32,
                                   kind="ExternalOutput").ap()
    with tile.TileContext(nc) as tc:
        with ExitStack() as ctx:
            _emit(nc, tc, ctx, dram, br2f)
    nc.compile()
    return nc


def kernel(x, v, L, W1, b1, W2, b2, Wr1, br1, Wr2, br2):
    x = np.ascontiguousarray(np.asarray(x, dtype=np.float32))
    v = np.ascontiguousarray(np.asarray(v, dtype=np.float32))
    consts, br2f = _build_consts(L, W1, b1, W2, b2, Wr1, br1, Wr2, br2)
    nc = _build_module(consts, br2f)

    from concourse.bass_utils import run_bass_kernel_spmd
    in_maps = []
    for c in range(NCORES):
        m = {"x": np.ascontiguousarray(x[c]), "v": np.ascontiguousarray(v[c])}
        m.update(consts)
        in_maps.append(m)
    import os as _os
    trace = _os.environ.get("KERNEL_TRACE", "0") == "1"
    tmpdir = _os.environ.get("KERNEL_TRACE_DIR") or None
    res = run_bass_kernel_spmd(nc, in_maps, core_ids=list(range(NCORES)),
                               trace=trace, tmpdir=tmpdir)
    global LAST_EXEC_TIME_NS, LAST_TRACE
    LAST_EXEC_TIME_NS = res.exec_time_ns
    LAST_TRACE = res.instructions_and_trace
    x_new = np.stack([r["x_new"] for r in res.results]).astype(np.float32)
    v_new = np.stack([r["v_new"] for r in res.results]).astype(np.float32)
    return (x_new, v_new)
